# revision 73
# baseline (speedup 1.0000x reference)
"""Trainium2 Bass kernel for LoFTR-style encoder layer (sparse attention + convs).

Sharding: pure data-parallel over batch B=8 -> 8 NeuronCores (one batch
element per core). BN statistics are all-reduced across cores, split per
128-channel half (o) so each collective hides behind the other half's
matmuls.

Device layout is channel-major ([C, spatial]); host does the (free)
transposes / weight reordering when staging inputs, and transposes the
per-core outputs back.

Structure vs the v1 kernel:
  - ft loaded s-major in 10 [128,1280] chunks so K/V projections start
    after the first two chunks instead of after the full 6.4 MB load.
  - attention j-loop software-pipelined (Qproj | S | message stages) and
    interleaved with the f-channel-only conv1 tiles (j=0..6, o=0).
  - the ki != m message matmuls (identically-zero bd blocks) are gone.
  - BN1/BN2 stats all-reduced per o-half; conv loops are o-outer so the
    o=0 collective overlaps the o=1 matmuls.
  - feat0 (bf16) stays resident in the ip1 f-tiles; the tail residual
    reads it from SBUF instead of re-loading ft from HBM.
  - memsets / big SBUF copies / BN normalize run on GpSimd; Z reciprocal
    uses the fast DVE approximation.

SBUF tag sharing (disjoint lifetimes):
  - tag "big":  ke, ve  ->  ip2_0, ip2_1
  - tag "b2" :  ftb0, ftb1  ->  y2_0, y2_1
"""

import os
import sys

import numpy as np

for _p in ("/opt/trn_rl_repo", os.path.expanduser("~/.axon_site/_ro/trn_rl_repo")):
    if os.path.isdir(_p) and _p not in sys.path:
        sys.path.insert(0, _p)

import ml_dtypes

import concourse.bass as bass
import concourse.mybir as mybir
import concourse.tile as tile
from concourse import bacc
from concourse.bass_utils import run_bass_kernel_spmd

F32 = mybir.dt.float32
BF16 = mybir.dt.bfloat16
AF = mybir.ActivationFunctionType
ALU = mybir.AluOpType

NCORES = 8
H = W = 80
HW = H * W          # 6400
D = 256
NI = 3200           # inside positions (image rows 0..39)
NHEAD = 8
PW = W + 2          # 82 padded width
ATTN_EPS = 1e-6
BN_EPS = 1e-5
BN_N = float(NCORES * HW)

# conv row-tiling: 5 output rows per psum tile (410 f32 cols <= 512/bank).
# NOTE: all conv psum allocations must keep the SAME written width — a
# narrower allocation followed by a wider one on the same rotating psC
# slot corrupts the columns beyond the narrow width (observed on HW).
CTILES = [(r0, 5) for r0 in range(0, 80, 5)]
NTI = len(CTILES)   # 16

# ft is loaded/cast in chunks of 1280 cols (16 image rows)
CHK = 1280
NCHK = HW // CHK    # 5 per m

# t-channel ip1 tiles cover padded rows 36..83 only (rows <41 are zero)
TR0 = 36
TROWS = 84 - TR0    # 48

LAST_EXEC_NS = None
LAST_MEAN_EXEC_NS = None

_cache = {}


def _bd(ap3):
    return ap3.rearrange("p a b -> p (a b)")


def _r3(ap2, a):
    return ap2.rearrange("p (a b) -> p a b", a=a)


def build_nc():
    nc = bacc.Bacc(
        "TRN2", target_bir_lowering=False, debug=False, num_devices=NCORES
    )

    ft_d = nc.dram_tensor("ft", [D, HW], BF16, kind="ExternalInput")
    wqt_d = nc.dram_tensor("wqt", [128, 2, D], BF16, kind="ExternalInput")
    wkt_d = nc.dram_tensor("wkt", [128, 2, D], BF16, kind="ExternalInput")
    wvt_d = nc.dram_tensor("wvt", [128, 2, D], BF16, kind="ExternalInput")
    # c1w rows 0..17: f-channel taps (ky*3+kx)*2+c; rows 18..35: t-channel taps
    c1w_d = nc.dram_tensor("c1w", [128, 36, D], BF16, kind="ExternalInput")
    c2w_d = nc.dram_tensor("c2w", [128, 18, D], BF16, kind="ExternalInput")
    bn1g_d = nc.dram_tensor("bn1g", [D, 1], F32, kind="ExternalInput")
    bn1b_d = nc.dram_tensor("bn1b", [D, 1], F32, kind="ExternalInput")
    bn2g_d = nc.dram_tensor("bn2g", [D, 1], F32, kind="ExternalInput")
    bn2b_d = nc.dram_tensor("bn2b", [D, 1], F32, kind="ExternalInput")
    mblk_d = nc.dram_tensor("mblk", [8, 256], BF16, kind="ExternalInput")
    out_d = nc.dram_tensor("out_t", [D, HW], BF16, kind="ExternalOutput")

    groups = [list(range(NCORES))]

    with tile.TileContext(nc) as tc:
        with (
            tc.tile_pool(name="pers", bufs=1) as pers,
            tc.tile_pool(name="bigp", bufs=2) as bigp,
            tc.tile_pool(name="b2p", bufs=2) as b2p,
            tc.tile_pool(name="y1p", bufs=2) as y1p,
            tc.tile_pool(name="qtp", bufs=6) as qtp,
            tc.tile_pool(name="scr", bufs=6) as scr,
            tc.tile_pool(name="small", bufs=1) as small,
            tc.tile_pool(name="fin", bufs=4) as fin,
            tc.tile_pool(name="psA", bufs=4, space="PSUM") as psA,
            tc.tile_pool(name="psS", bufs=1, space="PSUM") as psS,
            tc.tile_pool(name="psC", bufs=3, space="PSUM") as psC,
            tc.tile_pool(name="dram", bufs=1, space="DRAM") as dramp,
        ):
            # ---------------- persistent buffers ----------------
            wqt = pers.tile([128, 2, D], BF16, tag="wqt", name="wqt")
            wkt = pers.tile([128, 2, D], BF16, tag="wkt", name="wkt")
            wvt = pers.tile([128, 2, D], BF16, tag="wvt", name="wvt")
            c1w = pers.tile([128, 36, D], BF16, tag="c1w", name="c1w")
            c2w = pers.tile([128, 18, D], BF16, tag="c2w", name="c2w")
            maskblk = pers.tile([8, 256], BF16, tag="maskblk", name="maskblk")

            # conv1 input, f channels: [2 chunks][84 padded rows, 82 cols]
            # tile row r == padded row r == image row r-2. Also the tail's
            # resident copy of feat0 (bf16).
            ip1 = [
                pers.tile([128, 84, PW], BF16, tag=f"ip1_{c}", name=f"ip1_{c}")
                for c in range(2)
            ]
            # conv1 input, t channels: rows TR0..83 only (u = padded - TR0)
            ip1t = [
                pers.tile([128, TROWS, PW], BF16, tag=f"ip1t_{c}", name=f"ip1t_{c}")
                for c in range(2)
            ]
            bd = [
                pers.tile([128, 264], BF16, tag=f"bd{m}", name=f"bd{m}")
                for m in range(2)
            ]

            # zero-fill guards only (interiors are fully overwritten)
            def zero_guards(t, rows, wrow0, wrow1):
                # rows [0,wrow0) and [wrow1,rows) fully; cols 0 and 81 in between
                nc.gpsimd.memset(t[:, 0:wrow0, :], 0.0)
                nc.gpsimd.memset(t[:, wrow1:rows, :], 0.0)
                nc.gpsimd.memset(t[:, wrow0:wrow1, 0:1], 0.0)
                nc.gpsimd.memset(t[:, wrow0:wrow1, 81:82], 0.0)

            for c in range(2):
                zero_guards(ip1[c], 84, 2, 82)

            # ke/ve: [s-chunk part, i, d] (tag shared with ip2 later)
            ke = bigp.tile([128, 25, D], BF16, tag="big", name="ke")
            ve = bigp.tile([128, 25, D + 2], BF16, tag="big", name="ve")

            eps_t = small.tile([128, 1], F32, tag="eps_t", name="eps_t")
            nc.vector.memset(eps_t[:, :], BN_EPS)
            g1 = small.tile([128, 2], F32, tag="g1", name="g1")
            b1 = small.tile([128, 2], F32, tag="b1", name="b1")
            g2 = small.tile([128, 2], F32, tag="g2", name="g2")
            b2 = small.tile([128, 2], F32, tag="b2", name="b2")

            # ---------------- input DMAs + chunked cast ----------------
            ftb = [
                b2p.tile([128, HW], BF16, tag="b2", name=f"ftb{m}")
                for m in range(2)
            ]

            def load_chunk(s, m, eng=None):
                csl = slice(s * CHK, (s + 1) * CHK)
                (eng or nc.sync).dma_start(
                    ftb[m][:, csl],
                    ft_d[m * 128 : (m + 1) * 128, s * CHK : (s + 1) * CHK],
                )
                # mirror into the padded conv1 f-tile (16 image rows)
                nc.gpsimd.tensor_copy(
                    ip1[m][:, 2 + 16 * s : 18 + 16 * s, 1:81],
                    _r3(ftb[m][:, csl], 16),
                )

            nc.sync.dma_start(wkt[:, :, :], wkt_d[:, :, :])
            load_chunk(0, 0)
            load_chunk(0, 1)
            nc.sync.dma_start(wvt[:, :, :], wvt_d[:, :, :])
            load_chunk(1, 0)
            load_chunk(1, 1)
            nc.sync.dma_start(c1w[:, 0:18, :], c1w_d[:, 0:18, :])
            nc.sync.dma_start(wqt[:, :, :], wqt_d[:, :, :])
            load_chunk(2, 0)
            load_chunk(2, 1)
            nc.sync.dma_start(maskblk[:, :], mblk_d[:, :])
            load_chunk(3, 0)
            load_chunk(3, 1)
            nc.sync.dma_start(c1w[:, 18:36, :], c1w_d[:, 18:36, :])
            load_chunk(4, 0)
            load_chunk(4, 1)
            nc.sync.dma_start(c2w[:, :, :], c2w_d[:, :, :])
            # t-channel / KV guard zeroing (after f-copies on the gpsimd queue)
            nc.gpsimd.memset(ve[:, :, 0:1], 1.0)    # ones -> Ksum (m=0 band)
            nc.gpsimd.memset(ve[:, :, 257:258], 1.0)  # ones -> Ksum (m=1 band)
            for c in range(2):
                zero_guards(ip1t[c], TROWS, 6, 46)
            for m in range(2):
                nc.gpsimd.memset(bd[m][:, :], 0.0)
            for o in range(2):
                sl = slice(o * 128, (o + 1) * 128)
                nc.sync.dma_start(g1[:, o : o + 1], bn1g_d[sl, :])
                nc.sync.dma_start(b1[:, o : o + 1], bn1b_d[sl, :])
                nc.sync.dma_start(g2[:, o : o + 1], bn2g_d[sl, :])
                nc.sync.dma_start(b2[:, o : o + 1], bn2b_d[sl, :])

            # ---------------- K / V projections ([s, c] layout) ----------------
            # two s-chunks share one 512-col psum bank -> half the elu op count
            def kv_chunk(i, n):
                ps = psA.tile([128, 2 * D], F32, tag="psA", name="psA")
                for u in range(n):
                    for ki in range(2):
                        nc.tensor.matmul(
                            ps[:, u * D : (u + 1) * D],
                            ftb[ki][:, (i + u) * 128 : (i + u + 1) * 128],
                            wkt[:, ki, :],
                            start=(ki == 0),
                            stop=(ki == 1),
                        )
                # elu(x)+1 = relu(x) + exp(min(x,0))
                nd = n * D
                sm = scr.tile([128, 2 * D], F32, tag="scr", name="sm")
                se = scr.tile([128, 2 * D], F32, tag="scr", name="se")
                nc.vector.tensor_scalar_min(sm[:, :nd], ps[:, :nd], 0.0)
                nc.scalar.activation(se[:, :nd], sm[:, :nd], AF.Exp)
                nc.vector.scalar_tensor_tensor(
                    _bd(ke[:, i : i + n, :]), ps[:, :nd], 0.0, se[:, :nd],
                    ALU.max, ALU.add,
                )

                ps2 = psA.tile([128, 2 * D], F32, tag="psA", name="psA")
                for u in range(n):
                    for ki in range(2):
                        nc.tensor.matmul(
                            ps2[:, u * D : (u + 1) * D],
                            ftb[ki][:, (i + u) * 128 : (i + u + 1) * 128],
                            wvt[:, ki, :],
                            start=(ki == 0),
                            stop=(ki == 1),
                        )
                for u in range(n):
                    nc.scalar.copy(
                        ve[:, i + u, 1:257], ps2[:, u * D : (u + 1) * D]
                    )

            # ---------------- KV + Ksum -> block-diag BD ----------------
            def kvbd(m):
                # band m covers KV v-cols m*128..m*128+127 plus a ones col:
                # m=0: ve cols [0..129) (ones at 0); m=1: [129..258) (ones at 257)
                psm = psA.tile([128, 129], F32, tag="psA", name="psA")
                for i in range(25):
                    nc.tensor.matmul(
                        psm[:, :],
                        ke[:, i, m * 128 : (m + 1) * 128],
                        ve[:, i, 129 * m : 129 * m + 129],
                        start=(i == 0),
                        stop=(i == 24),
                    )
                blk0 = 1 - m          # psm col of first KV value in the band
                ks = 128 * m          # psm col of the ones/Ksum column
                for hh in range(4):
                    h = m * 4 + hh
                    lh = hh * 32
                    nc.vector.tensor_copy(
                        bd[m][lh : lh + 32, h * 32 : (h + 1) * 32],
                        psm[lh : lh + 32, blk0 + hh * 32 : blk0 + (hh + 1) * 32],
                    )
                    nc.vector.tensor_copy(
                        bd[m][lh : lh + 32, 256 + h : 257 + h],
                        psm[lh : lh + 32, ks : ks + 1],
                    )

            # ---------------- conv tile helpers ----------------
            y1 = [
                y1p.tile([128, H, PW], BF16, tag="y1", name=f"y1_{o}")
                for o in range(2)
            ]
            stats1 = [
                small.tile([128, 2 * NTI], F32, tag=f"stats1_{o}", name=f"stats1_{o}")
                for o in range(2)
            ]
            ip1f = [_bd(ip1[c][:, :, :]) for c in range(2)]
            ip1tf = [_bd(ip1t[c][:, :, :]) for c in range(2)]

            def conv_stats(j, o, ps, yt, stats, rt, nt):
                val = _r3(ps[:, :nt], rt)[:, :, 1:81]
                nc.vector.tensor_reduce(
                    stats[:, j : j + 1], val, mybir.AxisListType.XY, ALU.add,
                )
                sq = scr.tile([128, 492], F32, tag="scr", name="sq")
                nc.scalar.activation(
                    _r3(sq[:, : rt * 80], rt), val, AF.Square,
                    accum_out=stats[:, NTI + j : NTI + j + 1],
                )
                nc.vector.tensor_copy(
                    _bd(yt[:, :, :])[:, CTILES[j][0] * PW :
                                     CTILES[j][0] * PW + nt], ps[:, :nt]
                )

            def conv1_tile(j, o):
                r0, rt = CTILES[j]
                nt = rt * PW
                taps = []
                for c in range(2):
                    for ky in range(3):
                        for kx in range(3):
                            taps.append((c, ky, kx))
                for c in range(2, 4):
                    for ky in range(3):
                        if r0 + ky < 42 - rt:
                            continue  # t-channel rows all zero
                        for kx in range(3):
                            taps.append((c, ky, kx))
                ps = psC.tile([128, 492], F32, tag="psC", name="psC")
                for idx, (c, ky, kx) in enumerate(taps):
                    if c < 2:
                        s = (r0 + ky + 1) * PW + kx - 1
                        mv = ip1f[c][:, s : s + nt]
                        w = c1w[:, (ky * 3 + kx) * 2 + c, o * 128 : (o + 1) * 128]
                    else:
                        s = (r0 + ky + 1 - TR0) * PW + kx - 1
                        mv = ip1tf[c - 2][:, s : s + nt]
                        w = c1w[:, 18 + (ky * 3 + kx) * 2 + c - 2,
                                o * 128 : (o + 1) * 128]
                    nc.tensor.matmul(
                        ps[:, :nt],
                        w,
                        mv,
                        start=(idx == 0),
                        stop=(idx == len(taps) - 1),
                    )
                conv_stats(j, o, ps, y1[o], stats1[o], rt, nt)

            # ---------- attention j-stages ----------
            qt = {}

            def stage_A(j):  # Q^T projection + elu -> qt[j]
                lsl = slice(NI + j * 400, NI + (j + 1) * 400)
                qt[j] = [
                    qtp.tile([128, 400], BF16, tag="qteT", name=f"qt{j}_{m}")
                    for m in range(2)
                ]
                for m in range(2):
                    ps = psA.tile([128, 400], F32, tag="psA", name="psA")
                    for ki in range(2):
                        nc.tensor.matmul(
                            ps[:, :],
                            wqt[:, ki, m * 128 : (m + 1) * 128],
                            ftb[ki][:, lsl],
                            start=(ki == 0),
                            stop=(ki == 1),
                        )
                    sm = scr.tile([128, 400], F32, tag="scr", name="smq")
                    se = scr.tile([128, 400], F32, tag="scr", name="seq")
                    nc.vector.tensor_scalar_min(sm[:, :], ps[:, :], 0.0)
                    nc.scalar.activation(se[:, :], sm[:, :], AF.Exp)
                    nc.vector.scalar_tensor_tensor(
                        qt[j][m][:, :], ps[:, :], 0.0, se[:, :], ALU.max, ALU.add
                    )

            rs = {}

            def stage_B(j):  # S = Ksum . Q, Z = 1/(S+eps)
                pss = psS.tile([8, 400], F32, tag="psS", name="psS")
                for ki in range(2):
                    nc.tensor.matmul(
                        pss[:, :],
                        bd[ki][:, 256:264],
                        qt[j][ki][:, :],
                        start=(ki == 0),
                        stop=(ki == 1),
                    )
                sadd = scr.tile([128, 400], F32, tag="scr", name="sadd")
                rsf = scr.tile([128, 400], F32, tag="scr", name="rsf")
                rs[j] = qtp.tile([128, 400], BF16, tag="rsb", name=f"rs{j}")
                nc.vector.tensor_scalar_add(sadd[:8, :], pss[:, :], ATTN_EPS)
                nc.vector.reciprocal_approx_fast(rsf[:8, :], sadd[:8, :])
                nc.vector.tensor_copy(rs[j][:8, :], rsf[:8, :])

            def stage_C(j):  # message = (BD^T Q) * bcast(Z) -> ip1t
                for m in range(2):
                    psg = psA.tile([128, 400], F32, tag="psA", name="psA")
                    nc.tensor.matmul(
                        psg[:, :],
                        bd[m][:, m * 128 : (m + 1) * 128],
                        qt[j][m][:, :],
                    )
                    pre = psA.tile([128, 400], F32, tag="psA", name="psA")
                    nc.tensor.matmul(
                        pre[:, :], maskblk[:, m * 128 : (m + 1) * 128], rs[j][:8, :]
                    )
                    preb = scr.tile([128, 400], F32, tag="scr", name="preb")
                    nc.scalar.copy(preb[:, :], pre[:, :])
                    # l-tile j = image rows 40+5j..44+5j -> u rows 6+5j..
                    nc.vector.tensor_tensor(
                        ip1t[m][:, 6 + 5 * j : 11 + 5 * j, 1:81],
                        _r3(psg[:, :], 5),
                        _r3(preb[:, :], 5),
                        ALU.mult,
                    )

            # ---------- KV phase with f-only conv1 tiles as tensor filler ----------
            for i in range(0, 12, 2):
                kv_chunk(i, 2)
            conv1_tile(0, 0)
            for i in range(12, 18, 2):
                kv_chunk(i, 2)
            conv1_tile(1, 0)
            for i in range(18, 24, 2):
                kv_chunk(i, 2)
            kv_chunk(24, 1)
            # A(0)/A(1) early so their elu precedes the kvbd extraction
            # copies in the vector queue (B(0) needs bd + elu'd qt)
            stage_A(0)
            conv1_tile(0, 1)
            stage_A(1)
            kvbd(0)
            conv1_tile(1, 1)
            kvbd(1)
            stage_B(0)
            for t in range(2, 8):
                if t <= 6:
                    conv1_tile(t, 0)
                stage_A(t)
                if t <= 6:
                    conv1_tile(t, 1)
                stage_C(t - 2)
                stage_B(t - 1)
            stage_C(6)
            stage_B(7)
            stage_C(7)

            # ---------------- conv1 o=0 rest, then o=1 ----------------
            for j in range(7, NTI):
                conv1_tile(j, 0)

            # BN1 o=0 allreduce (overlaps conv1 o=1)
            def allreduce_start(statso, tag):
                bnst = small.tile([128, 2], F32, tag=f"bnst{tag}", name=f"bnst{tag}")
                nc.vector.tensor_reduce(
                    bnst[:, :],
                    statso[:, :].rearrange("p (k j) -> p k j", j=NTI),
                    mybir.AxisListType.X, ALU.add,
                )
                arin = dramp.tile([128, 2], F32, tag=f"ari{tag}", name=f"ari{tag}")
                arout = dramp.tile([128, 2], F32, tag=f"aro{tag}", name=f"aro{tag}")
                nc.scalar.dma_start(arin[:, :], bnst[:, :])
                nc.gpsimd.collective_compute(
                    "AllReduce", ALU.add, replica_groups=groups,
                    ins=[arin[:, :].opt()], outs=[arout[:, :].opt()],
                )
                gst = small.tile([128, 2], F32, tag=f"gst{tag}", name=f"gst{tag}")
                nc.scalar.dma_start(gst[:, :], arout[:, :])
                return gst

            gst1 = [None, None]
            gst1[0] = allreduce_start(stats1[0], "1a")

            def bn_coeffs(gst, gg, bb, o, tag, col=0):
                nm = small.tile([128, 1], F32, tag=f"nm{tag}", name=f"nm{tag}")
                ex2 = small.tile([128, 1], F32, tag=f"ex2{tag}", name=f"ex2{tag}")
                var = small.tile([128, 1], F32, tag=f"var{tag}", name=f"var{tag}")
                sd = small.tile([128, 1], F32, tag=f"sd{tag}", name=f"sd{tag}")
                rsd = small.tile([128, 1], F32, tag=f"rsd{tag}", name=f"rsd{tag}")
                scl = small.tile([128, 1], F32, tag=f"scl{tag}", name=f"scl{tag}")
                sh = small.tile([128, 1], F32, tag=f"sh{tag}", name=f"sh{tag}")
                nc.vector.tensor_scalar_mul(
                    nm[:, :], gst[:, col : col + 1], -1.0 / BN_N
                )
                nc.vector.tensor_scalar_mul(
                    ex2[:, :], gst[:, col + 1 : col + 2], 1.0 / BN_N
                )
                # var_neg = m^2 - E[x^2];  sd = sqrt(-var_neg + eps)
                nc.vector.scalar_tensor_tensor(
                    var[:, :], nm[:, :], nm[:, :], ex2[:, :],
                    ALU.mult, ALU.subtract,
                )
                nc.scalar.activation(
                    sd[:, :], var[:, :], AF.Sqrt, bias=eps_t[:, 0:1], scale=-1.0,
                )
                nc.vector.reciprocal(rsd[:, :], sd[:, :])
                nc.vector.tensor_tensor(
                    scl[:, :], rsd[:, :], gg[:, o : o + 1], ALU.mult
                )
                nc.vector.scalar_tensor_tensor(
                    sh[:, :], nm[:, :], scl[:, :], bb[:, o : o + 1],
                    ALU.mult, ALU.add,
                )
                return scl, sh

            # ip2 = BN1(y1) in padded layout (recycles ke/ve slots)
            ip2 = [
                bigp.tile([128, 84, PW], BF16, tag="big", name=f"ip2_{c}")
                for c in range(2)
            ]
            for c in range(2):
                zero_guards(ip2[c], 84, 2, 82)

            def bn1_norm(o, chunks):
                scl, sh = bn_coeffs(gst1[o], g1, b1, o, f"1{o}")
                rows = H // chunks
                for q in range(chunks):
                    r = rows * q
                    nc.gpsimd.tensor_scalar(
                        ip2[o][:, 2 + r : 2 + r + rows, 1:81],
                        y1[o][:, r : r + rows, 1:81],
                        scl[:, 0:1],
                        sh[:, 0:1],
                        ALU.mult,
                        ALU.add,
                    )

            # o=0 normalize overlaps the conv1 o=1 matmuls
            for j in range(7, NTI):
                if j == 9:
                    bn1_norm(0, 1)
                conv1_tile(j, 1)
            gst1[1] = allreduce_start(stats1[1], "1b")

            # ---------------- conv2 (+ stats), o-outer ----------------
            y2 = [
                b2p.tile([128, H, PW], BF16, tag="b2", name=f"y2_{o}")
                for o in range(2)
            ]
            stats2 = [
                small.tile([128, 2 * NTI], F32, tag=f"stats2_{o}", name=f"stats2_{o}")
                for o in range(2)
            ]
            ip2f = [_bd(ip2[c][:, :, :]) for c in range(2)]

            def conv2_taps(j, o, c, ps, start):
                r0, rt = CTILES[j]
                nt = rt * PW
                for i, (ky, kx) in enumerate(
                    [(ky, kx) for ky in range(3) for kx in range(3)]
                ):
                    s = (r0 + ky + 1) * PW + kx - 1
                    nc.tensor.matmul(
                        ps[:, :nt],
                        c2w[:, (ky * 3 + kx) * 2 + c, o * 128 : (o + 1) * 128],
                        ip2f[c][:, s : s + nt],
                        start=(start and i == 0),
                        stop=(i == 8),
                    )

            def conv2_tile(j, o):
                r0, rt = CTILES[j]
                nt = rt * PW
                ps = psC.tile([128, 492], F32, tag="psC", name="psC")
                idx = 0
                for c in range(2):
                    for ky in range(3):
                        for kx in range(3):
                            s = (r0 + ky + 1) * PW + kx - 1
                            nc.tensor.matmul(
                                ps[:, :nt],
                                c2w[:, (ky * 3 + kx) * 2 + c,
                                    o * 128 : (o + 1) * 128],
                                ip2f[c][:, s : s + nt],
                                start=(idx == 0),
                                stop=(idx == 17),
                            )
                            idx += 1
                conv_stats(j, o, ps, y2[o], stats2[o], rt, nt)

            sp2 = pers.tile([128, H, PW], BF16, tag="sp2", name="sp2")
            # o=1 spill recycles the y1_0 slot (dead once bn1_norm(0) ran)
            sp2b = y1p.tile([128, H, PW], BF16, tag="y1", name="sp2b")
            spill = [sp2, sp2b]

            def conv2_tile_c0(j, o):
                # c0-only partial (needs just ip2[0]); spill bf16
                r0, rt = CTILES[j]
                nt = rt * PW
                ps = psC.tile([128, 492], F32, tag="psC", name="psC")
                conv2_taps(j, o, 0, ps, start=True)
                nc.vector.tensor_copy(
                    _bd(spill[o][:, :, :])[:, r0 * PW : r0 * PW + nt], ps[:, :nt]
                )

            def conv2_tile_c1(j, o):
                # reload the c0 partial into psum, add the c1 taps
                r0, rt = CTILES[j]
                nt = rt * PW
                ps = psC.tile([128, 492], F32, tag="psC", name="psC")
                nc.scalar.copy(
                    ps[:, :nt], _bd(spill[o][:, :, :])[:, r0 * PW : r0 * PW + nt]
                )
                conv2_taps(j, o, 1, ps, start=False)
                conv_stats(j, o, ps, y2[o], stats2[o], rt, nt)

            # ---------------- BN2 + residual + store, per o ----------------
            def tail_chunk(o, k, scl2, sh2, gp):
                fsl = slice(800 * k, 800 * (k + 1))
                ysl = y2[o][:, 10 * k : 10 * (k + 1), 1:81]
                rsl = ip1[o][:, 2 + 10 * k : 12 + 10 * k, 1:81]
                tmp = fin.tile([128, 800], BF16, tag="tmp", name="tmp")
                ost = fin.tile([128, 800], BF16, tag="ost", name="ost")
                if gp:
                    nc.gpsimd.tensor_scalar(
                        _r3(tmp[:, :], 10), ysl, scl2[:, 0:1], sh2[:, 0:1],
                        ALU.mult, ALU.add,
                    )
                    nc.gpsimd.tensor_tensor(
                        _r3(ost[:, :], 10), _r3(tmp[:, :], 10), rsl, ALU.add,
                    )
                else:
                    nc.scalar.activation(
                        _r3(tmp[:, :], 10), ysl, AF.Identity,
                        bias=sh2[:, 0:1], scale=scl2[:, 0:1],
                    )
                    nc.vector.tensor_tensor(
                        _r3(ost[:, :], 10), _r3(tmp[:, :], 10), rsl, ALU.add,
                    )
                nc.sync.dma_start(out_d[o * 128 : (o + 1) * 128, fsl], ost[:, :])

            def tail(o, gst2, col, gp_mask):
                scl2, sh2 = bn_coeffs(gst2, g2, b2, o, f"2{o}", col=col)
                for k in range(8):
                    tail_chunk(o, k, scl2, sh2, gp_mask(k))

            # c0-only passes fill the BN1 o=1 allreduce window (need ip2[0] only)
            for j in range(NTI):
                conv2_tile_c0(j, 0)
            bn1_norm(1, 4)
            for j in range(NTI):
                conv2_tile_c0(j, 1)
            for j in range(NTI):
                conv2_tile_c1(j, 0)
            for j in range(NTI):
                conv2_tile_c1(j, 1)

            # single merged BN2 allreduce ([128,4]: o0 sum/sumsq, o1 sum/sumsq)
            bnst2 = small.tile([128, 4], F32, tag="bnst2", name="bnst2")
            for o in range(2):
                nc.vector.tensor_reduce(
                    bnst2[:, 2 * o : 2 * o + 2],
                    stats2[o][:, :].rearrange("p (k j) -> p k j", j=NTI),
                    mybir.AxisListType.X, ALU.add,
                )
            arin2 = dramp.tile([128, 4], F32, tag="ari2", name="ari2")
            arout2 = dramp.tile([128, 4], F32, tag="aro2", name="aro2")
            nc.scalar.dma_start(arin2[:, :], bnst2[:, :])
            nc.gpsimd.collective_compute(
                "AllReduce", ALU.add, replica_groups=groups,
                ins=[arin2[:, :].opt()], outs=[arout2[:, :].opt()],
            )
            gst2 = small.tile([128, 4], F32, tag="gst2", name="gst2")
            nc.scalar.dma_start(gst2[:, :], arout2[:, :])

            # both tails after the collective, split across engine lanes
            tail(0, gst2, 0, lambda k: k % 3 == 2)
            tail(1, gst2, 2, lambda k: k % 3 == 0)

    nc.compile()
    return nc


def _mblk():
    mb = np.zeros((8, 256), np.float32)
    for h in range(8):
        mb[h, h * 32 : (h + 1) * 32] = 1.0
    return mb.astype(ml_dtypes.bfloat16)


def _prep_inputs(feat0, zone_mask, w_q, w_k, w_v, conv1_w, bn1_g, bn1_b,
                 conv2_w, bn2_g, bn2_b, num_inside):
    B = feat0.shape[0]
    pos = np.asarray(zone_mask[:, :, 0])
    order = np.argsort(~pos, axis=1, kind="stable")
    assert np.array_equal(
        order[:, :num_inside],
        np.broadcast_to(np.arange(num_inside), (B, num_inside)),
    ), "kernel assumes inside positions are the first num_inside rows"
    assert num_inside == NI

    bf = ml_dtypes.bfloat16
    f32 = np.float32

    def wt(w):  # [dout, din] -> [128, 2, dout]: [p, ki, o] = w[o, ki*128+p]
        return np.ascontiguousarray(
            w.T.reshape(2, 128, D).transpose(1, 0, 2)
        ).astype(bf)

    def cw(w, nchunk):  # [O, I, 3, 3] -> [128, 9*nchunk, O]
        o_, i_, _, _ = w.shape
        r = w.transpose(2, 3, 1, 0).reshape(9, nchunk, 128, o_)
        return np.ascontiguousarray(
            r.transpose(2, 0, 1, 3).reshape(128, 9 * nchunk, o_)
        ).astype(bf)

    # conv1: f-channel taps (chunks 0,1) in rows 0..17, t-channel in 18..35
    c1 = np.asarray(conv1_w, f32)
    c1w_host = np.concatenate([cw(c1[:, :256], 2), cw(c1[:, 256:], 2)], axis=1)

    common = {
        "wqt": wt(np.asarray(w_q, f32)),
        "wkt": wt(np.asarray(w_k, f32)),
        "wvt": wt(np.asarray(w_v, f32)),
        "c1w": c1w_host,
        "c2w": cw(np.asarray(conv2_w, f32), 2),
        "bn1g": np.asarray(bn1_g, f32).reshape(D, 1),
        "bn1b": np.asarray(bn1_b, f32).reshape(D, 1),
        "bn2g": np.asarray(bn2_g, f32).reshape(D, 1),
        "bn2b": np.asarray(bn2_b, f32).reshape(D, 1),
        "mblk": _mblk(),
    }
    in_maps = []
    for b in range(NCORES):
        m = dict(common)
        m["ft"] = np.ascontiguousarray(np.asarray(feat0[b], f32).T).astype(bf)
        in_maps.append(m)
    return in_maps


def kernel(feat0, zone_mask, w_q, w_k, w_v, conv1_w, bn1_g, bn1_b,
           conv2_w, bn2_g, bn2_b, H=80, W=80, B=8, D=256, num_inside=3200,
           **_ignored):
    global LAST_EXEC_NS, LAST_MEAN_EXEC_NS
    if "nc" not in _cache:
        _cache["nc"] = build_nc()
    nc = _cache["nc"]

    in_maps = _prep_inputs(feat0, zone_mask, w_q, w_k, w_v, conv1_w, bn1_g,
                           bn1_b, conv2_w, bn2_g, bn2_b, int(num_inside))
    trace = os.environ.get("KERNEL_TRACE", "0") == "1"
    res = run_bass_kernel_spmd(nc, in_maps, list(range(NCORES)), trace=trace)
    LAST_EXEC_NS = res.exec_time_ns
    LAST_MEAN_EXEC_NS = res.mean_exec_time_ns
    out = np.empty((NCORES, HW, 256), np.float32)
    for b in range(NCORES):
        out[b] = res.results[b]["out_t"].T.astype(np.float32)
    return out


# revision 74
# speedup vs baseline: 1.0009x; 1.0009x over previous
"""Trainium2 Bass kernel for LoFTR-style encoder layer (sparse attention + convs).

Sharding: pure data-parallel over batch B=8 -> 8 NeuronCores (one batch
element per core). BN statistics are all-reduced across cores, split per
128-channel half (o) so each collective hides behind the other half's
matmuls.

Device layout is channel-major ([C, spatial]); host does the (free)
transposes / weight reordering when staging inputs, and transposes the
per-core outputs back.

Structure vs the v1 kernel:
  - ft loaded s-major in 10 [128,1280] chunks so K/V projections start
    after the first two chunks instead of after the full 6.4 MB load.
  - attention j-loop software-pipelined (Qproj | S | message stages) and
    interleaved with the f-channel-only conv1 tiles (j=0..6, o=0).
  - the ki != m message matmuls (identically-zero bd blocks) are gone.
  - BN1/BN2 stats all-reduced per o-half; conv loops are o-outer so the
    o=0 collective overlaps the o=1 matmuls.
  - feat0 (bf16) stays resident in the ip1 f-tiles; the tail residual
    reads it from SBUF instead of re-loading ft from HBM.
  - memsets / big SBUF copies / BN normalize run on GpSimd; Z reciprocal
    uses the fast DVE approximation.

SBUF tag sharing (disjoint lifetimes):
  - tag "big":  ke, ve  ->  ip2_0, ip2_1
  - tag "b2" :  ftb0, ftb1  ->  y2_0, y2_1
"""

import os
import sys

import numpy as np

for _p in ("/opt/trn_rl_repo", os.path.expanduser("~/.axon_site/_ro/trn_rl_repo")):
    if os.path.isdir(_p) and _p not in sys.path:
        sys.path.insert(0, _p)

import ml_dtypes

import concourse.bass as bass
import concourse.mybir as mybir
import concourse.tile as tile
from concourse import bacc
from concourse.bass_utils import run_bass_kernel_spmd

F32 = mybir.dt.float32
BF16 = mybir.dt.bfloat16
AF = mybir.ActivationFunctionType
ALU = mybir.AluOpType

NCORES = 8
H = W = 80
HW = H * W          # 6400
D = 256
NI = 3200           # inside positions (image rows 0..39)
NHEAD = 8
PW = W + 2          # 82 padded width
ATTN_EPS = 1e-6
BN_EPS = 1e-5
BN_N = float(NCORES * HW)

# conv row-tiling: 5 output rows per psum tile (410 f32 cols <= 512/bank).
# NOTE: all conv psum allocations must keep the SAME written width — a
# narrower allocation followed by a wider one on the same rotating psC
# slot corrupts the columns beyond the narrow width (observed on HW).
CTILES = [(r0, 5) for r0 in range(0, 80, 5)]
NTI = len(CTILES)   # 16

# ft is loaded/cast in chunks of 1280 cols (16 image rows)
CHK = 1280
NCHK = HW // CHK    # 5 per m

# t-channel ip1 tiles cover padded rows 36..83 only (rows <41 are zero)
TR0 = 36
TROWS = 84 - TR0    # 48

LAST_EXEC_NS = None
LAST_MEAN_EXEC_NS = None

_cache = {}


def _bd(ap3):
    return ap3.rearrange("p a b -> p (a b)")


def _r3(ap2, a):
    return ap2.rearrange("p (a b) -> p a b", a=a)


def build_nc():
    nc = bacc.Bacc(
        "TRN2", target_bir_lowering=False, debug=False, num_devices=NCORES
    )

    ft_d = nc.dram_tensor("ft", [D, HW], BF16, kind="ExternalInput")
    wqt_d = nc.dram_tensor("wqt", [128, 2, D], BF16, kind="ExternalInput")
    wkt_d = nc.dram_tensor("wkt", [128, 2, D], BF16, kind="ExternalInput")
    wvt_d = nc.dram_tensor("wvt", [128, 2, D], BF16, kind="ExternalInput")
    # c1w rows 0..17: f-channel taps (ky*3+kx)*2+c; rows 18..35: t-channel taps
    c1w_d = nc.dram_tensor("c1w", [128, 36, D], BF16, kind="ExternalInput")
    c2w_d = nc.dram_tensor("c2w", [128, 18, D], BF16, kind="ExternalInput")
    bn1g_d = nc.dram_tensor("bn1g", [D, 1], F32, kind="ExternalInput")
    bn1b_d = nc.dram_tensor("bn1b", [D, 1], F32, kind="ExternalInput")
    bn2g_d = nc.dram_tensor("bn2g", [D, 1], F32, kind="ExternalInput")
    bn2b_d = nc.dram_tensor("bn2b", [D, 1], F32, kind="ExternalInput")
    mblk_d = nc.dram_tensor("mblk", [8, 256], BF16, kind="ExternalInput")
    out_d = nc.dram_tensor("out_t", [D, HW], BF16, kind="ExternalOutput")

    groups = [list(range(NCORES))]

    with tile.TileContext(nc) as tc:
        with (
            tc.tile_pool(name="pers", bufs=1) as pers,
            tc.tile_pool(name="bigp", bufs=2) as bigp,
            tc.tile_pool(name="b2p", bufs=2) as b2p,
            tc.tile_pool(name="y1p", bufs=2) as y1p,
            tc.tile_pool(name="qtp", bufs=6) as qtp,
            tc.tile_pool(name="scr", bufs=6) as scr,
            tc.tile_pool(name="small", bufs=1) as small,
            tc.tile_pool(name="fin", bufs=2) as fin,
            tc.tile_pool(name="psA", bufs=4, space="PSUM") as psA,
            tc.tile_pool(name="psS", bufs=1, space="PSUM") as psS,
            tc.tile_pool(name="psC", bufs=3, space="PSUM") as psC,
            tc.tile_pool(name="dram", bufs=1, space="DRAM") as dramp,
        ):
            # ---------------- persistent buffers ----------------
            wqt = pers.tile([128, 2, D], BF16, tag="wqt", name="wqt")
            wkt = pers.tile([128, 2, D], BF16, tag="wkt", name="wkt")
            wvt = pers.tile([128, 2, D], BF16, tag="wvt", name="wvt")
            c1w = pers.tile([128, 36, D], BF16, tag="c1w", name="c1w")
            c2w = pers.tile([128, 18, D], BF16, tag="c2w", name="c2w")
            maskblk = pers.tile([8, 256], BF16, tag="maskblk", name="maskblk")

            # conv1 input, f channels: [2 chunks][84 padded rows, 82 cols]
            # tile row r == padded row r == image row r-2. Also the tail's
            # resident copy of feat0 (bf16).
            ip1 = [
                pers.tile([128, 84, PW], BF16, tag=f"ip1_{c}", name=f"ip1_{c}")
                for c in range(2)
            ]
            # conv1 input, t channels: rows TR0..83 only (u = padded - TR0)
            ip1t = [
                pers.tile([128, TROWS, PW], BF16, tag=f"ip1t_{c}", name=f"ip1t_{c}")
                for c in range(2)
            ]
            bd = [
                pers.tile([128, 264], BF16, tag=f"bd{m}", name=f"bd{m}")
                for m in range(2)
            ]

            # zero-fill guards only (interiors are fully overwritten)
            def zero_guards(t, rows, wrow0, wrow1):
                # rows [0,wrow0) and [wrow1,rows) fully; cols 0 and 81 in between
                nc.gpsimd.memset(t[:, 0:wrow0, :], 0.0)
                nc.gpsimd.memset(t[:, wrow1:rows, :], 0.0)
                nc.gpsimd.memset(t[:, wrow0:wrow1, 0:1], 0.0)
                nc.gpsimd.memset(t[:, wrow0:wrow1, 81:82], 0.0)

            for c in range(2):
                zero_guards(ip1[c], 84, 2, 82)

            # ke/ve: [s-chunk part, i, d] (tag shared with ip2 later)
            ke = bigp.tile([128, 25, D], BF16, tag="big", name="ke")
            ve = bigp.tile([128, 25, D + 2], BF16, tag="big", name="ve")

            eps_t = small.tile([128, 1], F32, tag="eps_t", name="eps_t")
            nc.vector.memset(eps_t[:, :], BN_EPS)
            g1 = small.tile([128, 2], F32, tag="g1", name="g1")
            b1 = small.tile([128, 2], F32, tag="b1", name="b1")
            g2 = small.tile([128, 2], F32, tag="g2", name="g2")
            b2 = small.tile([128, 2], F32, tag="b2", name="b2")

            # ---------------- input DMAs + chunked cast ----------------
            ftb = [
                b2p.tile([128, HW], BF16, tag="b2", name=f"ftb{m}")
                for m in range(2)
            ]

            def load_chunk(s, m, eng=None):
                csl = slice(s * CHK, (s + 1) * CHK)
                (eng or nc.sync).dma_start(
                    ftb[m][:, csl],
                    ft_d[m * 128 : (m + 1) * 128, s * CHK : (s + 1) * CHK],
                )
                # mirror into the padded conv1 f-tile (16 image rows)
                nc.gpsimd.tensor_copy(
                    ip1[m][:, 2 + 16 * s : 18 + 16 * s, 1:81],
                    _r3(ftb[m][:, csl], 16),
                )

            nc.sync.dma_start(wkt[:, :, :], wkt_d[:, :, :])
            load_chunk(0, 0)
            load_chunk(0, 1)
            nc.sync.dma_start(wvt[:, :, :], wvt_d[:, :, :])
            load_chunk(1, 0)
            load_chunk(1, 1)
            nc.sync.dma_start(c1w[:, 0:18, :], c1w_d[:, 0:18, :])
            nc.sync.dma_start(wqt[:, :, :], wqt_d[:, :, :])
            load_chunk(2, 0)
            load_chunk(2, 1)
            nc.sync.dma_start(maskblk[:, :], mblk_d[:, :])
            load_chunk(3, 0)
            load_chunk(3, 1)
            nc.sync.dma_start(c1w[:, 18:36, :], c1w_d[:, 18:36, :])
            load_chunk(4, 0)
            load_chunk(4, 1)
            nc.sync.dma_start(c2w[:, :, :], c2w_d[:, :, :])
            # t-channel / KV guard zeroing (after f-copies on the gpsimd queue)
            nc.gpsimd.memset(ve[:, :, 0:1], 1.0)    # ones -> Ksum (m=0 band)
            nc.gpsimd.memset(ve[:, :, 257:258], 1.0)  # ones -> Ksum (m=1 band)
            for c in range(2):
                zero_guards(ip1t[c], TROWS, 6, 46)
            for m in range(2):
                nc.gpsimd.memset(bd[m][:, :], 0.0)
            for o in range(2):
                sl = slice(o * 128, (o + 1) * 128)
                nc.sync.dma_start(g1[:, o : o + 1], bn1g_d[sl, :])
                nc.sync.dma_start(b1[:, o : o + 1], bn1b_d[sl, :])
                nc.sync.dma_start(g2[:, o : o + 1], bn2g_d[sl, :])
                nc.sync.dma_start(b2[:, o : o + 1], bn2b_d[sl, :])

            # ---------------- K / V projections ([s, c] layout) ----------------
            # two s-chunks share one 512-col psum bank -> half the elu op count
            def kv_chunk(i, n):
                ps = psA.tile([128, 2 * D], F32, tag="psA", name="psA")
                for u in range(n):
                    for ki in range(2):
                        nc.tensor.matmul(
                            ps[:, u * D : (u + 1) * D],
                            ftb[ki][:, (i + u) * 128 : (i + u + 1) * 128],
                            wkt[:, ki, :],
                            start=(ki == 0),
                            stop=(ki == 1),
                        )
                # elu(x)+1 = relu(x) + exp(min(x,0))
                nd = n * D
                sm = scr.tile([128, 2 * D], F32, tag="scr", name="sm")
                se = scr.tile([128, 2 * D], F32, tag="scr", name="se")
                nc.vector.tensor_scalar_min(sm[:, :nd], ps[:, :nd], 0.0)
                nc.scalar.activation(se[:, :nd], sm[:, :nd], AF.Exp)
                nc.vector.scalar_tensor_tensor(
                    _bd(ke[:, i : i + n, :]), ps[:, :nd], 0.0, se[:, :nd],
                    ALU.max, ALU.add,
                )

                ps2 = psA.tile([128, 2 * D], F32, tag="psA", name="psA")
                for u in range(n):
                    for ki in range(2):
                        nc.tensor.matmul(
                            ps2[:, u * D : (u + 1) * D],
                            ftb[ki][:, (i + u) * 128 : (i + u + 1) * 128],
                            wvt[:, ki, :],
                            start=(ki == 0),
                            stop=(ki == 1),
                        )
                for u in range(n):
                    nc.scalar.copy(
                        ve[:, i + u, 1:257], ps2[:, u * D : (u + 1) * D]
                    )

            # ---------------- KV + Ksum -> block-diag BD ----------------
            def kvbd(m):
                # band m covers KV v-cols m*128..m*128+127 plus a ones col:
                # m=0: ve cols [0..129) (ones at 0); m=1: [129..258) (ones at 257)
                psm = psA.tile([128, 129], F32, tag="psA", name="psA")
                for i in range(25):
                    nc.tensor.matmul(
                        psm[:, :],
                        ke[:, i, m * 128 : (m + 1) * 128],
                        ve[:, i, 129 * m : 129 * m + 129],
                        start=(i == 0),
                        stop=(i == 24),
                    )
                blk0 = 1 - m          # psm col of first KV value in the band
                ks = 128 * m          # psm col of the ones/Ksum column
                for hh in range(4):
                    h = m * 4 + hh
                    lh = hh * 32
                    nc.vector.tensor_copy(
                        bd[m][lh : lh + 32, h * 32 : (h + 1) * 32],
                        psm[lh : lh + 32, blk0 + hh * 32 : blk0 + (hh + 1) * 32],
                    )
                    nc.vector.tensor_copy(
                        bd[m][lh : lh + 32, 256 + h : 257 + h],
                        psm[lh : lh + 32, ks : ks + 1],
                    )

            # ---------------- conv tile helpers ----------------
            y1 = [
                y1p.tile([128, H, PW], BF16, tag="y1", name=f"y1_{o}")
                for o in range(2)
            ]
            stats1 = [
                small.tile([128, 2 * NTI], F32, tag=f"stats1_{o}", name=f"stats1_{o}")
                for o in range(2)
            ]
            ip1f = [_bd(ip1[c][:, :, :]) for c in range(2)]
            ip1tf = [_bd(ip1t[c][:, :, :]) for c in range(2)]

            def conv_stats(j, o, ps, yt, stats, rt, nt):
                val = _r3(ps[:, :nt], rt)[:, :, 1:81]
                nc.vector.tensor_reduce(
                    stats[:, j : j + 1], val, mybir.AxisListType.XY, ALU.add,
                )
                sq = scr.tile([128, 492], F32, tag="scr", name="sq")
                nc.scalar.activation(
                    _r3(sq[:, : rt * 80], rt), val, AF.Square,
                    accum_out=stats[:, NTI + j : NTI + j + 1],
                )
                nc.vector.tensor_copy(
                    _bd(yt[:, :, :])[:, CTILES[j][0] * PW :
                                     CTILES[j][0] * PW + nt], ps[:, :nt]
                )

            def conv1_tile(j, o):
                r0, rt = CTILES[j]
                nt = rt * PW
                taps = []
                for c in range(2):
                    for ky in range(3):
                        for kx in range(3):
                            taps.append((c, ky, kx))
                for c in range(2, 4):
                    for ky in range(3):
                        if r0 + ky < 42 - rt:
                            continue  # t-channel rows all zero
                        for kx in range(3):
                            taps.append((c, ky, kx))
                ps = psC.tile([128, 492], F32, tag="psC", name="psC")
                for idx, (c, ky, kx) in enumerate(taps):
                    if c < 2:
                        s = (r0 + ky + 1) * PW + kx - 1
                        mv = ip1f[c][:, s : s + nt]
                        w = c1w[:, (ky * 3 + kx) * 2 + c, o * 128 : (o + 1) * 128]
                    else:
                        s = (r0 + ky + 1 - TR0) * PW + kx - 1
                        mv = ip1tf[c - 2][:, s : s + nt]
                        w = c1w[:, 18 + (ky * 3 + kx) * 2 + c - 2,
                                o * 128 : (o + 1) * 128]
                    nc.tensor.matmul(
                        ps[:, :nt],
                        w,
                        mv,
                        start=(idx == 0),
                        stop=(idx == len(taps) - 1),
                    )
                conv_stats(j, o, ps, y1[o], stats1[o], rt, nt)

            # ---------- attention j-stages ----------
            qt = {}

            def stage_A(j):  # Q^T projection + elu -> qt[j]
                lsl = slice(NI + j * 400, NI + (j + 1) * 400)
                qt[j] = [
                    qtp.tile([128, 400], BF16, tag="qteT", name=f"qt{j}_{m}")
                    for m in range(2)
                ]
                for m in range(2):
                    ps = psA.tile([128, 400], F32, tag="psA", name="psA")
                    for ki in range(2):
                        nc.tensor.matmul(
                            ps[:, :],
                            wqt[:, ki, m * 128 : (m + 1) * 128],
                            ftb[ki][:, lsl],
                            start=(ki == 0),
                            stop=(ki == 1),
                        )
                    sm = scr.tile([128, 400], F32, tag="scr", name="smq")
                    se = scr.tile([128, 400], F32, tag="scr", name="seq")
                    nc.vector.tensor_scalar_min(sm[:, :], ps[:, :], 0.0)
                    nc.scalar.activation(se[:, :], sm[:, :], AF.Exp)
                    nc.vector.scalar_tensor_tensor(
                        qt[j][m][:, :], ps[:, :], 0.0, se[:, :], ALU.max, ALU.add
                    )

            rs = {}

            def stage_B(j):  # S = Ksum . Q, Z = 1/(S+eps)
                pss = psS.tile([8, 400], F32, tag="psS", name="psS")
                for ki in range(2):
                    nc.tensor.matmul(
                        pss[:, :],
                        bd[ki][:, 256:264],
                        qt[j][ki][:, :],
                        start=(ki == 0),
                        stop=(ki == 1),
                    )
                sadd = scr.tile([128, 400], F32, tag="scr", name="sadd")
                rsf = scr.tile([128, 400], F32, tag="scr", name="rsf")
                rs[j] = qtp.tile([128, 400], BF16, tag="rsb", name=f"rs{j}")
                nc.vector.tensor_scalar_add(sadd[:8, :], pss[:, :], ATTN_EPS)
                nc.vector.reciprocal_approx_fast(rsf[:8, :], sadd[:8, :])
                nc.vector.tensor_copy(rs[j][:8, :], rsf[:8, :])

            def stage_C(j):  # message = (BD^T Q) * bcast(Z) -> ip1t
                for m in range(2):
                    psg = psA.tile([128, 400], F32, tag="psA", name="psA")
                    nc.tensor.matmul(
                        psg[:, :],
                        bd[m][:, m * 128 : (m + 1) * 128],
                        qt[j][m][:, :],
                    )
                    pre = psA.tile([128, 400], F32, tag="psA", name="psA")
                    nc.tensor.matmul(
                        pre[:, :], maskblk[:, m * 128 : (m + 1) * 128], rs[j][:8, :]
                    )
                    preb = scr.tile([128, 400], F32, tag="scr", name="preb")
                    nc.scalar.copy(preb[:, :], pre[:, :])
                    # l-tile j = image rows 40+5j..44+5j -> u rows 6+5j..
                    nc.vector.tensor_tensor(
                        ip1t[m][:, 6 + 5 * j : 11 + 5 * j, 1:81],
                        _r3(psg[:, :], 5),
                        _r3(preb[:, :], 5),
                        ALU.mult,
                    )

            # ---------- KV phase with f-only conv1 tiles as tensor filler ----------
            for i in range(0, 12, 2):
                kv_chunk(i, 2)
            conv1_tile(0, 0)
            for i in range(12, 18, 2):
                kv_chunk(i, 2)
            conv1_tile(1, 0)
            for i in range(18, 24, 2):
                kv_chunk(i, 2)
            kv_chunk(24, 1)
            # A(0)/A(1) early so their elu precedes the kvbd extraction
            # copies in the vector queue (B(0) needs bd + elu'd qt)
            stage_A(0)
            conv1_tile(0, 1)
            stage_A(1)
            kvbd(0)
            conv1_tile(1, 1)
            kvbd(1)
            stage_B(0)
            for t in range(2, 8):
                if t <= 6:
                    conv1_tile(t, 0)
                stage_A(t)
                if t <= 6:
                    conv1_tile(t, 1)
                stage_C(t - 2)
                stage_B(t - 1)
            stage_C(6)
            stage_B(7)
            stage_C(7)

            # ---------------- conv1 o=0 rest, then o=1 ----------------
            for j in range(7, NTI):
                conv1_tile(j, 0)

            # BN1 o=0 allreduce (overlaps conv1 o=1)
            def allreduce_start(statso, tag):
                bnst = small.tile([128, 2], F32, tag=f"bnst{tag}", name=f"bnst{tag}")
                nc.vector.tensor_reduce(
                    bnst[:, :],
                    statso[:, :].rearrange("p (k j) -> p k j", j=NTI),
                    mybir.AxisListType.X, ALU.add,
                )
                arin = dramp.tile([128, 2], F32, tag=f"ari{tag}", name=f"ari{tag}")
                arout = dramp.tile([128, 2], F32, tag=f"aro{tag}", name=f"aro{tag}")
                nc.scalar.dma_start(arin[:, :], bnst[:, :])
                nc.gpsimd.collective_compute(
                    "AllReduce", ALU.add, replica_groups=groups,
                    ins=[arin[:, :].opt()], outs=[arout[:, :].opt()],
                )
                gst = small.tile([128, 2], F32, tag=f"gst{tag}", name=f"gst{tag}")
                nc.scalar.dma_start(gst[:, :], arout[:, :])
                return gst

            gst1 = [None, None]
            gst1[0] = allreduce_start(stats1[0], "1a")

            def bn_coeffs(gst, gg, bb, o, tag, col=0):
                nm = small.tile([128, 1], F32, tag=f"nm{tag}", name=f"nm{tag}")
                ex2 = small.tile([128, 1], F32, tag=f"ex2{tag}", name=f"ex2{tag}")
                var = small.tile([128, 1], F32, tag=f"var{tag}", name=f"var{tag}")
                sd = small.tile([128, 1], F32, tag=f"sd{tag}", name=f"sd{tag}")
                rsd = small.tile([128, 1], F32, tag=f"rsd{tag}", name=f"rsd{tag}")
                scl = small.tile([128, 1], F32, tag=f"scl{tag}", name=f"scl{tag}")
                sh = small.tile([128, 1], F32, tag=f"sh{tag}", name=f"sh{tag}")
                nc.vector.tensor_scalar_mul(
                    nm[:, :], gst[:, col : col + 1], -1.0 / BN_N
                )
                nc.vector.tensor_scalar_mul(
                    ex2[:, :], gst[:, col + 1 : col + 2], 1.0 / BN_N
                )
                # var_neg = m^2 - E[x^2];  sd = sqrt(-var_neg + eps)
                nc.vector.scalar_tensor_tensor(
                    var[:, :], nm[:, :], nm[:, :], ex2[:, :],
                    ALU.mult, ALU.subtract,
                )
                nc.scalar.activation(
                    sd[:, :], var[:, :], AF.Sqrt, bias=eps_t[:, 0:1], scale=-1.0,
                )
                nc.vector.reciprocal(rsd[:, :], sd[:, :])
                nc.vector.tensor_tensor(
                    scl[:, :], rsd[:, :], gg[:, o : o + 1], ALU.mult
                )
                nc.vector.scalar_tensor_tensor(
                    sh[:, :], nm[:, :], scl[:, :], bb[:, o : o + 1],
                    ALU.mult, ALU.add,
                )
                return scl, sh

            # ip2 = BN1(y1) in padded layout (recycles ke/ve slots)
            ip2 = [
                bigp.tile([128, 84, PW], BF16, tag="big", name=f"ip2_{c}")
                for c in range(2)
            ]
            for c in range(2):
                zero_guards(ip2[c], 84, 2, 82)

            def bn1_norm(o, chunks):
                scl, sh = bn_coeffs(gst1[o], g1, b1, o, f"1{o}")
                rows = H // chunks
                for q in range(chunks):
                    r = rows * q
                    nc.gpsimd.tensor_scalar(
                        ip2[o][:, 2 + r : 2 + r + rows, 1:81],
                        y1[o][:, r : r + rows, 1:81],
                        scl[:, 0:1],
                        sh[:, 0:1],
                        ALU.mult,
                        ALU.add,
                    )

            # o=0 normalize overlaps the conv1 o=1 matmuls
            for j in range(7, NTI):
                if j == 9:
                    bn1_norm(0, 1)
                conv1_tile(j, 1)
            gst1[1] = allreduce_start(stats1[1], "1b")

            # ---------------- conv2 (+ stats), o-outer ----------------
            y2 = [
                b2p.tile([128, H, PW], BF16, tag="b2", name=f"y2_{o}")
                for o in range(2)
            ]
            stats2 = [
                small.tile([128, 2 * NTI], F32, tag=f"stats2_{o}", name=f"stats2_{o}")
                for o in range(2)
            ]
            ip2f = [_bd(ip2[c][:, :, :]) for c in range(2)]

            def conv2_taps(j, o, c, ps, start):
                r0, rt = CTILES[j]
                nt = rt * PW
                for i, (ky, kx) in enumerate(
                    [(ky, kx) for ky in range(3) for kx in range(3)]
                ):
                    s = (r0 + ky + 1) * PW + kx - 1
                    nc.tensor.matmul(
                        ps[:, :nt],
                        c2w[:, (ky * 3 + kx) * 2 + c, o * 128 : (o + 1) * 128],
                        ip2f[c][:, s : s + nt],
                        start=(start and i == 0),
                        stop=(i == 8),
                    )

            def conv2_tile(j, o):
                r0, rt = CTILES[j]
                nt = rt * PW
                ps = psC.tile([128, 492], F32, tag="psC", name="psC")
                idx = 0
                for c in range(2):
                    for ky in range(3):
                        for kx in range(3):
                            s = (r0 + ky + 1) * PW + kx - 1
                            nc.tensor.matmul(
                                ps[:, :nt],
                                c2w[:, (ky * 3 + kx) * 2 + c,
                                    o * 128 : (o + 1) * 128],
                                ip2f[c][:, s : s + nt],
                                start=(idx == 0),
                                stop=(idx == 17),
                            )
                            idx += 1
                conv_stats(j, o, ps, y2[o], stats2[o], rt, nt)

            sp2 = pers.tile([128, H, PW], BF16, tag="sp2", name="sp2")
            # o=1 spill recycles the y1_0 slot (dead once bn1_norm(0) ran)
            sp2b = y1p.tile([128, H, PW], BF16, tag="y1", name="sp2b")
            spill = [sp2, sp2b]

            def conv2_tile_c0(j, o):
                # c0-only partial (needs just ip2[0]); spill bf16
                r0, rt = CTILES[j]
                nt = rt * PW
                ps = psC.tile([128, 492], F32, tag="psC", name="psC")
                conv2_taps(j, o, 0, ps, start=True)
                nc.vector.tensor_copy(
                    _bd(spill[o][:, :, :])[:, r0 * PW : r0 * PW + nt], ps[:, :nt]
                )

            def conv2_tile_c1(j, o):
                # reload the c0 partial into psum, add the c1 taps
                r0, rt = CTILES[j]
                nt = rt * PW
                ps = psC.tile([128, 492], F32, tag="psC", name="psC")
                nc.scalar.copy(
                    ps[:, :nt], _bd(spill[o][:, :, :])[:, r0 * PW : r0 * PW + nt]
                )
                conv2_taps(j, o, 1, ps, start=False)
                conv_stats(j, o, ps, y2[o], stats2[o], rt, nt)

            # ---------------- BN2 + residual + store, per o ----------------
            def tail_chunk(o, k, scl2, sh2, gp):
                fsl = slice(1600 * k, 1600 * (k + 1))
                ysl = y2[o][:, 20 * k : 20 * (k + 1), 1:81]
                rsl = ip1[o][:, 2 + 20 * k : 22 + 20 * k, 1:81]
                tmp = fin.tile([128, 1600], BF16, tag="tmp", name="tmp")
                ost = fin.tile([128, 1600], BF16, tag="ost", name="ost")
                if gp:
                    nc.gpsimd.tensor_scalar(
                        _r3(tmp[:, :], 20), ysl, scl2[:, 0:1], sh2[:, 0:1],
                        ALU.mult, ALU.add,
                    )
                    nc.gpsimd.tensor_tensor(
                        _r3(ost[:, :], 20), _r3(tmp[:, :], 20), rsl, ALU.add,
                    )
                else:
                    nc.scalar.activation(
                        _r3(tmp[:, :], 20), ysl, AF.Identity,
                        bias=sh2[:, 0:1], scale=scl2[:, 0:1],
                    )
                    nc.vector.tensor_tensor(
                        _r3(ost[:, :], 20), _r3(tmp[:, :], 20), rsl, ALU.add,
                    )
                nc.sync.dma_start(out_d[o * 128 : (o + 1) * 128, fsl], ost[:, :])

            def tail(o, gst2, col, gp_mask):
                scl2, sh2 = bn_coeffs(gst2, g2, b2, o, f"2{o}", col=col)
                for k in range(4):
                    tail_chunk(o, k, scl2, sh2, gp_mask(k))

            # c0-only passes fill the BN1 o=1 allreduce window (need ip2[0] only)
            for j in range(NTI):
                conv2_tile_c0(j, 0)
            bn1_norm(1, 4)
            for j in range(NTI):
                conv2_tile_c0(j, 1)
            for j in range(NTI):
                conv2_tile_c1(j, 0)
            for j in range(NTI):
                conv2_tile_c1(j, 1)

            # single merged BN2 allreduce ([128,4]: o0 sum/sumsq, o1 sum/sumsq)
            bnst2 = small.tile([128, 4], F32, tag="bnst2", name="bnst2")
            for o in range(2):
                nc.vector.tensor_reduce(
                    bnst2[:, 2 * o : 2 * o + 2],
                    stats2[o][:, :].rearrange("p (k j) -> p k j", j=NTI),
                    mybir.AxisListType.X, ALU.add,
                )
            arin2 = dramp.tile([128, 4], F32, tag="ari2", name="ari2")
            arout2 = dramp.tile([128, 4], F32, tag="aro2", name="aro2")
            nc.scalar.dma_start(arin2[:, :], bnst2[:, :])
            nc.gpsimd.collective_compute(
                "AllReduce", ALU.add, replica_groups=groups,
                ins=[arin2[:, :].opt()], outs=[arout2[:, :].opt()],
            )
            gst2 = small.tile([128, 4], F32, tag="gst2", name="gst2")
            nc.scalar.dma_start(gst2[:, :], arout2[:, :])

            # both tails after the collective, split across engine lanes
            tail(0, gst2, 0, lambda k: k == 2)
            tail(1, gst2, 2, lambda k: k == 1)

    nc.compile()
    return nc


def _mblk():
    mb = np.zeros((8, 256), np.float32)
    for h in range(8):
        mb[h, h * 32 : (h + 1) * 32] = 1.0
    return mb.astype(ml_dtypes.bfloat16)


def _prep_inputs(feat0, zone_mask, w_q, w_k, w_v, conv1_w, bn1_g, bn1_b,
                 conv2_w, bn2_g, bn2_b, num_inside):
    B = feat0.shape[0]
    pos = np.asarray(zone_mask[:, :, 0])
    order = np.argsort(~pos, axis=1, kind="stable")
    assert np.array_equal(
        order[:, :num_inside],
        np.broadcast_to(np.arange(num_inside), (B, num_inside)),
    ), "kernel assumes inside positions are the first num_inside rows"
    assert num_inside == NI

    bf = ml_dtypes.bfloat16
    f32 = np.float32

    def wt(w):  # [dout, din] -> [128, 2, dout]: [p, ki, o] = w[o, ki*128+p]
        return np.ascontiguousarray(
            w.T.reshape(2, 128, D).transpose(1, 0, 2)
        ).astype(bf)

    def cw(w, nchunk):  # [O, I, 3, 3] -> [128, 9*nchunk, O]
        o_, i_, _, _ = w.shape
        r = w.transpose(2, 3, 1, 0).reshape(9, nchunk, 128, o_)
        return np.ascontiguousarray(
            r.transpose(2, 0, 1, 3).reshape(128, 9 * nchunk, o_)
        ).astype(bf)

    # conv1: f-channel taps (chunks 0,1) in rows 0..17, t-channel in 18..35
    c1 = np.asarray(conv1_w, f32)
    c1w_host = np.concatenate([cw(c1[:, :256], 2), cw(c1[:, 256:], 2)], axis=1)

    common = {
        "wqt": wt(np.asarray(w_q, f32)),
        "wkt": wt(np.asarray(w_k, f32)),
        "wvt": wt(np.asarray(w_v, f32)),
        "c1w": c1w_host,
        "c2w": cw(np.asarray(conv2_w, f32), 2),
        "bn1g": np.asarray(bn1_g, f32).reshape(D, 1),
        "bn1b": np.asarray(bn1_b, f32).reshape(D, 1),
        "bn2g": np.asarray(bn2_g, f32).reshape(D, 1),
        "bn2b": np.asarray(bn2_b, f32).reshape(D, 1),
        "mblk": _mblk(),
    }
    in_maps = []
    for b in range(NCORES):
        m = dict(common)
        m["ft"] = np.ascontiguousarray(np.asarray(feat0[b], f32).T).astype(bf)
        in_maps.append(m)
    return in_maps


def kernel(feat0, zone_mask, w_q, w_k, w_v, conv1_w, bn1_g, bn1_b,
           conv2_w, bn2_g, bn2_b, H=80, W=80, B=8, D=256, num_inside=3200,
           **_ignored):
    global LAST_EXEC_NS, LAST_MEAN_EXEC_NS
    if "nc" not in _cache:
        _cache["nc"] = build_nc()
    nc = _cache["nc"]

    in_maps = _prep_inputs(feat0, zone_mask, w_q, w_k, w_v, conv1_w, bn1_g,
                           bn1_b, conv2_w, bn2_g, bn2_b, int(num_inside))
    trace = os.environ.get("KERNEL_TRACE", "0") == "1"
    res = run_bass_kernel_spmd(nc, in_maps, list(range(NCORES)), trace=trace)
    LAST_EXEC_NS = res.exec_time_ns
    LAST_MEAN_EXEC_NS = res.mean_exec_time_ns
    out = np.empty((NCORES, HW, 256), np.float32)
    for b in range(NCORES):
        out[b] = res.results[b]["out_t"].T.astype(np.float32)
    return out


# revision 75
# speedup vs baseline: 1.0199x; 1.0191x over previous
"""Trainium2 Bass kernel for LoFTR-style encoder layer (sparse attention + convs).

Sharding: pure data-parallel over batch B=8 -> 8 NeuronCores (one batch
element per core). BN statistics are all-reduced across cores, split per
128-channel half (o) so each collective hides behind the other half's
matmuls.

Device layout is channel-major ([C, spatial]); host does the (free)
transposes / weight reordering when staging inputs, and transposes the
per-core outputs back.

Structure vs the v1 kernel:
  - ft loaded s-major in 10 [128,1280] chunks so K/V projections start
    after the first two chunks instead of after the full 6.4 MB load.
  - attention j-loop software-pipelined (Qproj | S | message stages) and
    interleaved with the f-channel-only conv1 tiles (j=0..6, o=0).
  - the ki != m message matmuls (identically-zero bd blocks) are gone.
  - BN1/BN2 stats all-reduced per o-half; conv loops are o-outer so the
    o=0 collective overlaps the o=1 matmuls.
  - feat0 (bf16) stays resident in the ip1 f-tiles; the tail residual
    reads it from SBUF instead of re-loading ft from HBM.
  - memsets / big SBUF copies / BN normalize run on GpSimd; Z reciprocal
    uses the fast DVE approximation.

SBUF tag sharing (disjoint lifetimes):
  - tag "big":  ke, ve  ->  ip2_0, ip2_1
  - tag "b2" :  ftb0, ftb1  ->  y2_0, y2_1
"""

import os
import sys

import numpy as np

for _p in ("/opt/trn_rl_repo", os.path.expanduser("~/.axon_site/_ro/trn_rl_repo")):
    if os.path.isdir(_p) and _p not in sys.path:
        sys.path.insert(0, _p)

import ml_dtypes

import concourse.bass as bass
import concourse.mybir as mybir
import concourse.tile as tile
from concourse import bacc
from concourse.bass_utils import run_bass_kernel_spmd

F32 = mybir.dt.float32
BF16 = mybir.dt.bfloat16
AF = mybir.ActivationFunctionType
ALU = mybir.AluOpType

NCORES = 8
H = W = 80
HW = H * W          # 6400
D = 256
NI = 3200           # inside positions (image rows 0..39)
NHEAD = 8
PW = W + 2          # 82 padded width
ATTN_EPS = 1e-6
BN_EPS = 1e-5
BN_N = float(NCORES * HW)

# conv row-tiling: 5 output rows per psum tile (410 f32 cols <= 512/bank).
# NOTE: all conv psum allocations must keep the SAME written width — a
# narrower allocation followed by a wider one on the same rotating psC
# slot corrupts the columns beyond the narrow width (observed on HW).
CTILES = [(r0, 5) for r0 in range(0, 80, 5)]
NTI = len(CTILES)   # 16

# ft is loaded/cast in chunks of 1280 cols (16 image rows)
CHK = 1280
NCHK = HW // CHK    # 5 per m

# t-channel ip1 tiles cover padded rows 36..83 only (rows <41 are zero)
TR0 = 36
TROWS = 84 - TR0    # 48

LAST_EXEC_NS = None
LAST_MEAN_EXEC_NS = None

_cache = {}


def _bd(ap3):
    return ap3.rearrange("p a b -> p (a b)")


def _r3(ap2, a):
    return ap2.rearrange("p (a b) -> p a b", a=a)


def build_nc():
    nc = bacc.Bacc(
        "TRN2", target_bir_lowering=False, debug=False, num_devices=NCORES
    )

    ft_d = nc.dram_tensor("ft", [D, HW], BF16, kind="ExternalInput")
    wqt_d = nc.dram_tensor("wqt", [128, 2, D], BF16, kind="ExternalInput")
    wkt_d = nc.dram_tensor("wkt", [128, 2, D], BF16, kind="ExternalInput")
    wvt_d = nc.dram_tensor("wvt", [128, 2, D], BF16, kind="ExternalInput")
    # c1w rows 0..17: f-channel taps (ky*3+kx)*2+c; rows 18..35: t-channel taps
    c1w_d = nc.dram_tensor("c1w", [128, 36, D], BF16, kind="ExternalInput")
    c2w_d = nc.dram_tensor("c2w", [128, 18, D], BF16, kind="ExternalInput")
    bn1g_d = nc.dram_tensor("bn1g", [D, 1], F32, kind="ExternalInput")
    bn1b_d = nc.dram_tensor("bn1b", [D, 1], F32, kind="ExternalInput")
    bn2g_d = nc.dram_tensor("bn2g", [D, 1], F32, kind="ExternalInput")
    bn2b_d = nc.dram_tensor("bn2b", [D, 1], F32, kind="ExternalInput")
    mblk_d = nc.dram_tensor("mblk", [8, 256], BF16, kind="ExternalInput")
    out_d = nc.dram_tensor("out_t", [D, HW], BF16, kind="ExternalOutput")

    groups = [list(range(NCORES))]

    with tile.TileContext(nc) as tc:
        with (
            tc.tile_pool(name="pers", bufs=1) as pers,
            tc.tile_pool(name="bigp", bufs=2) as bigp,
            tc.tile_pool(name="b2p", bufs=2) as b2p,
            tc.tile_pool(name="y1p", bufs=2) as y1p,
            tc.tile_pool(name="qtp", bufs=6) as qtp,
            tc.tile_pool(name="scr", bufs=6) as scr,
            tc.tile_pool(name="small", bufs=1) as small,
            tc.tile_pool(name="fin", bufs=4) as fin,
            tc.tile_pool(name="psA", bufs=4, space="PSUM") as psA,
            tc.tile_pool(name="psS", bufs=1, space="PSUM") as psS,
            tc.tile_pool(name="psC", bufs=3, space="PSUM") as psC,
            tc.tile_pool(name="dram", bufs=1, space="DRAM") as dramp,
        ):
            # ---------------- persistent buffers ----------------
            wqt = pers.tile([128, 2, D], BF16, tag="wqt", name="wqt")
            wkt = pers.tile([128, 2, D], BF16, tag="wkt", name="wkt")
            wvt = pers.tile([128, 2, D], BF16, tag="wvt", name="wvt")
            c1w = pers.tile([128, 36, D], BF16, tag="c1w", name="c1w")
            c2w = pers.tile([128, 18, D], BF16, tag="c2w", name="c2w")
            maskblk = pers.tile([8, 256], BF16, tag="maskblk", name="maskblk")

            # conv1 input, f channels: [2 chunks][84 padded rows, 82 cols]
            # tile row r == padded row r == image row r-2. Also the tail's
            # resident copy of feat0 (bf16).
            ip1 = [
                pers.tile([128, 84, PW], BF16, tag=f"ip1_{c}", name=f"ip1_{c}")
                for c in range(2)
            ]
            # conv1 input, t channels: rows TR0..83 only (u = padded - TR0)
            ip1t = [
                pers.tile([128, TROWS, PW], BF16, tag=f"ip1t_{c}", name=f"ip1t_{c}")
                for c in range(2)
            ]
            bd = [
                pers.tile([128, 264], BF16, tag=f"bd{m}", name=f"bd{m}")
                for m in range(2)
            ]

            # zero-fill guards only (interiors are fully overwritten)
            def zero_guards(t, rows, wrow0, wrow1):
                # rows [0,wrow0) and [wrow1,rows) fully; cols 0 and 81 in between
                nc.gpsimd.memset(t[:, 0:wrow0, :], 0.0)
                nc.gpsimd.memset(t[:, wrow1:rows, :], 0.0)
                nc.gpsimd.memset(t[:, wrow0:wrow1, 0:1], 0.0)
                nc.gpsimd.memset(t[:, wrow0:wrow1, 81:82], 0.0)

            for c in range(2):
                zero_guards(ip1[c], 84, 2, 82)

            # ke/ve: [s-chunk part, i, d] (tag shared with ip2 later)
            ke = bigp.tile([128, 25, D], BF16, tag="big", name="ke")
            ve = bigp.tile([128, 25, D + 2], BF16, tag="big", name="ve")

            eps_t = small.tile([128, 1], F32, tag="eps_t", name="eps_t")
            nc.vector.memset(eps_t[:, :], BN_EPS)
            g1 = small.tile([128, 2], F32, tag="g1", name="g1")
            b1 = small.tile([128, 2], F32, tag="b1", name="b1")
            g2 = small.tile([128, 2], F32, tag="g2", name="g2")
            b2 = small.tile([128, 2], F32, tag="b2", name="b2")

            # ---------------- input DMAs + chunked cast ----------------
            ftb = [
                b2p.tile([128, HW], BF16, tag="b2", name=f"ftb{m}")
                for m in range(2)
            ]

            def load_chunk(s, m, eng=None):
                csl = slice(s * CHK, (s + 1) * CHK)
                (eng or nc.sync).dma_start(
                    ftb[m][:, csl],
                    ft_d[m * 128 : (m + 1) * 128, s * CHK : (s + 1) * CHK],
                )
                # mirror into the padded conv1 f-tile (16 image rows)
                nc.gpsimd.tensor_copy(
                    ip1[m][:, 2 + 16 * s : 18 + 16 * s, 1:81],
                    _r3(ftb[m][:, csl], 16),
                )

            nc.sync.dma_start(wkt[:, :, :], wkt_d[:, :, :])
            load_chunk(0, 0)
            load_chunk(0, 1)
            nc.sync.dma_start(wvt[:, :, :], wvt_d[:, :, :])
            load_chunk(1, 0)
            load_chunk(1, 1)
            nc.sync.dma_start(c1w[:, 0:18, :], c1w_d[:, 0:18, :])
            nc.sync.dma_start(wqt[:, :, :], wqt_d[:, :, :])
            load_chunk(2, 0)
            load_chunk(2, 1)
            nc.sync.dma_start(maskblk[:, :], mblk_d[:, :])
            load_chunk(3, 0)
            load_chunk(3, 1)
            nc.sync.dma_start(c1w[:, 18:36, :], c1w_d[:, 18:36, :])
            load_chunk(4, 0)
            load_chunk(4, 1)
            nc.sync.dma_start(c2w[:, :, :], c2w_d[:, :, :])
            # t-channel / KV guard zeroing (after f-copies on the gpsimd queue)
            nc.gpsimd.memset(ve[:, :, 0:1], 1.0)    # ones -> Ksum (m=0 band)
            nc.gpsimd.memset(ve[:, :, 257:258], 1.0)  # ones -> Ksum (m=1 band)
            for c in range(2):
                zero_guards(ip1t[c], TROWS, 6, 46)
            for m in range(2):
                nc.gpsimd.memset(bd[m][:, :], 0.0)
            for o in range(2):
                sl = slice(o * 128, (o + 1) * 128)
                nc.sync.dma_start(g1[:, o : o + 1], bn1g_d[sl, :])
                nc.sync.dma_start(b1[:, o : o + 1], bn1b_d[sl, :])
                nc.sync.dma_start(g2[:, o : o + 1], bn2g_d[sl, :])
                nc.sync.dma_start(b2[:, o : o + 1], bn2b_d[sl, :])

            # ---------------- K / V projections ([s, c] layout) ----------------
            # two s-chunks share one 512-col psum bank -> half the elu op count
            def kv_chunk(i, n):
                ps = psA.tile([128, 2 * D], F32, tag="psA", name="psA")
                for u in range(n):
                    for ki in range(2):
                        nc.tensor.matmul(
                            ps[:, u * D : (u + 1) * D],
                            ftb[ki][:, (i + u) * 128 : (i + u + 1) * 128],
                            wkt[:, ki, :],
                            start=(ki == 0),
                            stop=(ki == 1),
                        )
                # elu(x)+1 = relu(x) + exp(min(x,0))
                nd = n * D
                sm = scr.tile([128, 2 * D], F32, tag="scr", name="sm")
                se = scr.tile([128, 2 * D], F32, tag="scr", name="se")
                nc.vector.tensor_scalar_min(sm[:, :nd], ps[:, :nd], 0.0)
                nc.scalar.activation(se[:, :nd], sm[:, :nd], AF.Exp)
                nc.vector.scalar_tensor_tensor(
                    _bd(ke[:, i : i + n, :]), ps[:, :nd], 0.0, se[:, :nd],
                    ALU.max, ALU.add,
                )

                ps2 = psA.tile([128, 2 * D], F32, tag="psA", name="psA")
                for u in range(n):
                    for ki in range(2):
                        nc.tensor.matmul(
                            ps2[:, u * D : (u + 1) * D],
                            ftb[ki][:, (i + u) * 128 : (i + u + 1) * 128],
                            wvt[:, ki, :],
                            start=(ki == 0),
                            stop=(ki == 1),
                        )
                for u in range(n):
                    nc.scalar.copy(
                        ve[:, i + u, 1:257], ps2[:, u * D : (u + 1) * D]
                    )

            # ---------------- KV + Ksum -> block-diag BD ----------------
            def kvbd(m):
                # band m covers KV v-cols m*128..m*128+127 plus a ones col:
                # m=0: ve cols [0..129) (ones at 0); m=1: [129..258) (ones at 257)
                psm = psA.tile([128, 129], F32, tag="psA", name="psA")
                for i in range(25):
                    nc.tensor.matmul(
                        psm[:, :],
                        ke[:, i, m * 128 : (m + 1) * 128],
                        ve[:, i, 129 * m : 129 * m + 129],
                        start=(i == 0),
                        stop=(i == 24),
                    )
                blk0 = 1 - m          # psm col of first KV value in the band
                ks = 128 * m          # psm col of the ones/Ksum column
                for hh in range(4):
                    h = m * 4 + hh
                    lh = hh * 32
                    nc.vector.tensor_copy(
                        bd[m][lh : lh + 32, h * 32 : (h + 1) * 32],
                        psm[lh : lh + 32, blk0 + hh * 32 : blk0 + (hh + 1) * 32],
                    )
                    nc.vector.tensor_copy(
                        bd[m][lh : lh + 32, 256 + h : 257 + h],
                        psm[lh : lh + 32, ks : ks + 1],
                    )

            # ---------------- conv tile helpers ----------------
            y1 = [
                y1p.tile([128, H, PW], BF16, tag="y1", name=f"y1_{o}")
                for o in range(2)
            ]
            stats1 = [
                small.tile([128, 2 * NTI], F32, tag=f"stats1_{o}", name=f"stats1_{o}")
                for o in range(2)
            ]
            ip1f = [_bd(ip1[c][:, :, :]) for c in range(2)]
            ip1tf = [_bd(ip1t[c][:, :, :]) for c in range(2)]

            def conv_stats(j, o, ps, yt, stats, rt, nt):
                val = _r3(ps[:, :nt], rt)[:, :, 1:81]
                nc.vector.tensor_reduce(
                    stats[:, j : j + 1], val, mybir.AxisListType.XY, ALU.add,
                )
                sq = scr.tile([128, 492], F32, tag="scr", name="sq")
                nc.scalar.activation(
                    _r3(sq[:, : rt * 80], rt), val, AF.Square,
                    accum_out=stats[:, NTI + j : NTI + j + 1],
                )
                nc.vector.tensor_copy(
                    _bd(yt[:, :, :])[:, CTILES[j][0] * PW :
                                     CTILES[j][0] * PW + nt], ps[:, :nt]
                )

            def conv1_tile(j, o):
                r0, rt = CTILES[j]
                nt = rt * PW
                taps = []
                for c in range(2):
                    for ky in range(3):
                        for kx in range(3):
                            taps.append((c, ky, kx))
                for c in range(2, 4):
                    for ky in range(3):
                        if r0 + ky < 42 - rt:
                            continue  # t-channel rows all zero
                        for kx in range(3):
                            taps.append((c, ky, kx))
                ps = psC.tile([128, 492], F32, tag="psC", name="psC")
                for idx, (c, ky, kx) in enumerate(taps):
                    if c < 2:
                        s = (r0 + ky + 1) * PW + kx - 1
                        mv = ip1f[c][:, s : s + nt]
                        w = c1w[:, (ky * 3 + kx) * 2 + c, o * 128 : (o + 1) * 128]
                    else:
                        s = (r0 + ky + 1 - TR0) * PW + kx - 1
                        mv = ip1tf[c - 2][:, s : s + nt]
                        w = c1w[:, 18 + (ky * 3 + kx) * 2 + c - 2,
                                o * 128 : (o + 1) * 128]
                    nc.tensor.matmul(
                        ps[:, :nt],
                        w,
                        mv,
                        start=(idx == 0),
                        stop=(idx == len(taps) - 1),
                    )
                conv_stats(j, o, ps, y1[o], stats1[o], rt, nt)

            # ---------- attention j-stages ----------
            qt = {}

            def stage_A(j):  # Q^T projection + elu -> qt[j]
                lsl = slice(NI + j * 400, NI + (j + 1) * 400)
                qt[j] = [
                    qtp.tile([128, 400], BF16, tag="qteT", name=f"qt{j}_{m}")
                    for m in range(2)
                ]
                for m in range(2):
                    ps = psA.tile([128, 400], F32, tag="psA", name="psA")
                    for ki in range(2):
                        nc.tensor.matmul(
                            ps[:, :],
                            wqt[:, ki, m * 128 : (m + 1) * 128],
                            ftb[ki][:, lsl],
                            start=(ki == 0),
                            stop=(ki == 1),
                        )
                    sm = scr.tile([128, 400], F32, tag="scr", name="smq")
                    se = scr.tile([128, 400], F32, tag="scr", name="seq")
                    nc.vector.tensor_scalar_min(sm[:, :], ps[:, :], 0.0)
                    nc.scalar.activation(se[:, :], sm[:, :], AF.Exp)
                    nc.vector.scalar_tensor_tensor(
                        qt[j][m][:, :], ps[:, :], 0.0, se[:, :], ALU.max, ALU.add
                    )

            rs = {}

            def stage_B(j):  # S = Ksum . Q, Z = 1/(S+eps)
                pss = psS.tile([8, 400], F32, tag="psS", name="psS")
                for ki in range(2):
                    nc.tensor.matmul(
                        pss[:, :],
                        bd[ki][:, 256:264],
                        qt[j][ki][:, :],
                        start=(ki == 0),
                        stop=(ki == 1),
                    )
                sadd = scr.tile([128, 400], F32, tag="scr", name="sadd")
                rsf = scr.tile([128, 400], F32, tag="scr", name="rsf")
                rs[j] = qtp.tile([128, 400], BF16, tag="rsb", name=f"rs{j}")
                nc.vector.tensor_scalar_add(sadd[:8, :], pss[:, :], ATTN_EPS)
                nc.vector.reciprocal_approx_fast(rsf[:8, :], sadd[:8, :])
                nc.vector.tensor_copy(rs[j][:8, :], rsf[:8, :])

            def stage_C(j):  # message = (BD^T Q) * bcast(Z) -> ip1t
                for m in range(2):
                    psg = psA.tile([128, 400], F32, tag="psA", name="psA")
                    nc.tensor.matmul(
                        psg[:, :],
                        bd[m][:, m * 128 : (m + 1) * 128],
                        qt[j][m][:, :],
                    )
                    pre = psA.tile([128, 400], F32, tag="psA", name="psA")
                    nc.tensor.matmul(
                        pre[:, :], maskblk[:, m * 128 : (m + 1) * 128], rs[j][:8, :]
                    )
                    preb = scr.tile([128, 400], F32, tag="scr", name="preb")
                    nc.scalar.copy(preb[:, :], pre[:, :])
                    # l-tile j = image rows 40+5j..44+5j -> u rows 6+5j..
                    nc.vector.tensor_tensor(
                        ip1t[m][:, 6 + 5 * j : 11 + 5 * j, 1:81],
                        _r3(psg[:, :], 5),
                        _r3(preb[:, :], 5),
                        ALU.mult,
                    )

            # ---------- KV phase with f-only conv1 tiles as tensor filler ----------
            for i in range(0, 12, 2):
                kv_chunk(i, 2)
            conv1_tile(0, 0)
            for i in range(12, 18, 2):
                kv_chunk(i, 2)
            conv1_tile(1, 0)
            for i in range(18, 24, 2):
                kv_chunk(i, 2)
            kv_chunk(24, 1)
            # A(0)/A(1) early so their elu precedes the kvbd extraction
            # copies in the vector queue (B(0) needs bd + elu'd qt)
            stage_A(0)
            conv1_tile(0, 1)
            stage_A(1)
            kvbd(0)
            conv1_tile(1, 1)
            kvbd(1)
            stage_B(0)
            for t in range(2, 8):
                if t <= 6:
                    conv1_tile(t, 0)
                stage_A(t)
                if t <= 6:
                    conv1_tile(t, 1)
                stage_C(t - 2)
                stage_B(t - 1)
            stage_C(6)
            stage_B(7)
            stage_C(7)

            # ---------------- conv1 o=0 rest, then o=1 ----------------
            for j in range(7, NTI):
                conv1_tile(j, 0)

            # BN1 o=0 allreduce (overlaps conv1 o=1)
            def allreduce_start(statso, tag):
                bnst = small.tile([128, 2], F32, tag=f"bnst{tag}", name=f"bnst{tag}")
                nc.vector.tensor_reduce(
                    bnst[:, :],
                    statso[:, :].rearrange("p (k j) -> p k j", j=NTI),
                    mybir.AxisListType.X, ALU.add,
                )
                arin = dramp.tile([128, 2], F32, tag=f"ari{tag}", name=f"ari{tag}")
                arout = dramp.tile([128, 2], F32, tag=f"aro{tag}", name=f"aro{tag}")
                nc.scalar.dma_start(arin[:, :], bnst[:, :])
                nc.gpsimd.collective_compute(
                    "AllReduce", ALU.add, replica_groups=groups,
                    ins=[arin[:, :].opt()], outs=[arout[:, :].opt()],
                )
                gst = small.tile([128, 2], F32, tag=f"gst{tag}", name=f"gst{tag}")
                nc.scalar.dma_start(gst[:, :], arout[:, :])
                return gst

            gst1 = [None, None]
            gst1[0] = allreduce_start(stats1[0], "1a")

            def bn_coeffs(gst, gg, bb, o, tag, col=0):
                nm = small.tile([128, 1], F32, tag=f"nm{tag}", name=f"nm{tag}")
                ex2 = small.tile([128, 1], F32, tag=f"ex2{tag}", name=f"ex2{tag}")
                var = small.tile([128, 1], F32, tag=f"var{tag}", name=f"var{tag}")
                sd = small.tile([128, 1], F32, tag=f"sd{tag}", name=f"sd{tag}")
                rsd = small.tile([128, 1], F32, tag=f"rsd{tag}", name=f"rsd{tag}")
                scl = small.tile([128, 1], F32, tag=f"scl{tag}", name=f"scl{tag}")
                sh = small.tile([128, 1], F32, tag=f"sh{tag}", name=f"sh{tag}")
                nc.vector.tensor_scalar_mul(
                    nm[:, :], gst[:, col : col + 1], -1.0 / BN_N
                )
                nc.vector.tensor_scalar_mul(
                    ex2[:, :], gst[:, col + 1 : col + 2], 1.0 / BN_N
                )
                # var_neg = m^2 - E[x^2];  sd = sqrt(-var_neg + eps)
                nc.vector.scalar_tensor_tensor(
                    var[:, :], nm[:, :], nm[:, :], ex2[:, :],
                    ALU.mult, ALU.subtract,
                )
                nc.scalar.activation(
                    sd[:, :], var[:, :], AF.Sqrt, bias=eps_t[:, 0:1], scale=-1.0,
                )
                nc.vector.reciprocal(rsd[:, :], sd[:, :])
                nc.vector.tensor_tensor(
                    scl[:, :], rsd[:, :], gg[:, o : o + 1], ALU.mult
                )
                nc.vector.scalar_tensor_tensor(
                    sh[:, :], nm[:, :], scl[:, :], bb[:, o : o + 1],
                    ALU.mult, ALU.add,
                )
                return scl, sh

            # ip2 = BN1(y1) in padded layout (recycles ke/ve slots)
            ip2 = [
                bigp.tile([128, 84, PW], BF16, tag="big", name=f"ip2_{c}")
                for c in range(2)
            ]
            for c in range(2):
                zero_guards(ip2[c], 84, 2, 82)

            def bn1_norm(o, chunks):
                scl, sh = bn_coeffs(gst1[o], g1, b1, o, f"1{o}")
                rows = H // chunks
                for q in range(chunks):
                    r = rows * q
                    nc.gpsimd.tensor_scalar(
                        ip2[o][:, 2 + r : 2 + r + rows, 1:81],
                        y1[o][:, r : r + rows, 1:81],
                        scl[:, 0:1],
                        sh[:, 0:1],
                        ALU.mult,
                        ALU.add,
                    )

            # o=0 normalize overlaps the conv1 o=1 matmuls
            for j in range(7, NTI):
                if j == 9:
                    bn1_norm(0, 1)
                conv1_tile(j, 1)
            gst1[1] = allreduce_start(stats1[1], "1b")

            # ---------------- conv2 (+ stats), o-outer ----------------
            y2 = [
                b2p.tile([128, H, PW], BF16, tag="b2", name=f"y2_{o}")
                for o in range(2)
            ]
            stats2 = [
                small.tile([128, 2 * NTI], F32, tag=f"stats2_{o}", name=f"stats2_{o}")
                for o in range(2)
            ]
            ip2f = [_bd(ip2[c][:, :, :]) for c in range(2)]

            def conv2_taps(j, o, c, ps, start):
                r0, rt = CTILES[j]
                nt = rt * PW
                for i, (ky, kx) in enumerate(
                    [(ky, kx) for ky in range(3) for kx in range(3)]
                ):
                    s = (r0 + ky + 1) * PW + kx - 1
                    nc.tensor.matmul(
                        ps[:, :nt],
                        c2w[:, (ky * 3 + kx) * 2 + c, o * 128 : (o + 1) * 128],
                        ip2f[c][:, s : s + nt],
                        start=(start and i == 0),
                        stop=(i == 8),
                    )

            def conv2_tile(j, o):
                r0, rt = CTILES[j]
                nt = rt * PW
                ps = psC.tile([128, 492], F32, tag="psC", name="psC")
                idx = 0
                for c in range(2):
                    for ky in range(3):
                        for kx in range(3):
                            s = (r0 + ky + 1) * PW + kx - 1
                            nc.tensor.matmul(
                                ps[:, :nt],
                                c2w[:, (ky * 3 + kx) * 2 + c,
                                    o * 128 : (o + 1) * 128],
                                ip2f[c][:, s : s + nt],
                                start=(idx == 0),
                                stop=(idx == 17),
                            )
                            idx += 1
                conv_stats(j, o, ps, y2[o], stats2[o], rt, nt)

            sp2 = pers.tile([128, H, PW], BF16, tag="sp2", name="sp2")
            # o=1 spill recycles the y1_0 slot (dead once bn1_norm(0) ran)
            sp2b = y1p.tile([128, H, PW], BF16, tag="y1", name="sp2b")
            spill = [sp2, sp2b]

            def conv2_tile_c0(j, o):
                # c0-only partial (needs just ip2[0]); spill bf16
                r0, rt = CTILES[j]
                nt = rt * PW
                ps = psC.tile([128, 492], F32, tag="psC", name="psC")
                conv2_taps(j, o, 0, ps, start=True)
                nc.vector.tensor_copy(
                    _bd(spill[o][:, :, :])[:, r0 * PW : r0 * PW + nt], ps[:, :nt]
                )

            def conv2_tile_c1(j, o):
                # reload the c0 partial into psum, add the c1 taps
                r0, rt = CTILES[j]
                nt = rt * PW
                ps = psC.tile([128, 492], F32, tag="psC", name="psC")
                nc.scalar.copy(
                    ps[:, :nt], _bd(spill[o][:, :, :])[:, r0 * PW : r0 * PW + nt]
                )
                conv2_taps(j, o, 1, ps, start=False)
                conv_stats(j, o, ps, y2[o], stats2[o], rt, nt)

            # ---------------- BN2 + residual + store, per o ----------------
            def tail_chunk(o, k, scl2, sh2, gp):
                fsl = slice(800 * k, 800 * (k + 1))
                ysl = y2[o][:, 10 * k : 10 * (k + 1), 1:81]
                rsl = ip1[o][:, 2 + 10 * k : 12 + 10 * k, 1:81]
                tmp = fin.tile([128, 800], BF16, tag="tmp", name="tmp")
                ost = fin.tile([128, 800], BF16, tag="ost", name="ost")
                if gp:
                    nc.gpsimd.tensor_scalar(
                        _r3(tmp[:, :], 10), ysl, scl2[:, 0:1], sh2[:, 0:1],
                        ALU.mult, ALU.add,
                    )
                    nc.gpsimd.tensor_tensor(
                        _r3(ost[:, :], 10), _r3(tmp[:, :], 10), rsl, ALU.add,
                    )
                else:
                    nc.scalar.activation(
                        _r3(tmp[:, :], 10), ysl, AF.Identity,
                        bias=sh2[:, 0:1], scale=scl2[:, 0:1],
                    )
                    nc.vector.tensor_tensor(
                        _r3(ost[:, :], 10), _r3(tmp[:, :], 10), rsl, ALU.add,
                    )
                nc.sync.dma_start(out_d[o * 128 : (o + 1) * 128, fsl], ost[:, :])

            def tail(o, gst2, col, gp_mask):
                scl2, sh2 = bn_coeffs(gst2, g2, b2, o, f"2{o}", col=col)
                for k in range(8):
                    tail_chunk(o, k, scl2, sh2, gp_mask(k))

            # c0-only passes fill the BN1 o=1 allreduce window (need ip2[0] only)
            for j in range(NTI):
                conv2_tile_c0(j, 0)
            bn1_norm(1, 4)
            for j in range(NTI):
                conv2_tile_c0(j, 1)
            for j in range(NTI):
                conv2_tile_c1(j, 0)
            for j in range(NTI):
                conv2_tile_c1(j, 1)

            # single merged BN2 allreduce ([128,4]: o0 sum/sumsq, o1 sum/sumsq)
            bnst2 = small.tile([128, 4], F32, tag="bnst2", name="bnst2")
            for o in range(2):
                nc.vector.tensor_reduce(
                    bnst2[:, 2 * o : 2 * o + 2],
                    stats2[o][:, :].rearrange("p (k j) -> p k j", j=NTI),
                    mybir.AxisListType.X, ALU.add,
                )
            arin2 = dramp.tile([128, 4], F32, tag="ari2", name="ari2")
            arout2 = dramp.tile([128, 4], F32, tag="aro2", name="aro2")
            nc.scalar.dma_start(arin2[:, :], bnst2[:, :])
            nc.gpsimd.collective_compute(
                "AllReduce", ALU.add, replica_groups=groups,
                ins=[arin2[:, :].opt()], outs=[arout2[:, :].opt()],
            )
            gst2 = small.tile([128, 4], F32, tag="gst2", name="gst2")
            nc.scalar.dma_start(gst2[:, :], arout2[:, :])

            # both tails after the collective, split across engine lanes
            tail(0, gst2, 0, lambda k: k % 3 == 2)
            tail(1, gst2, 2, lambda k: k % 3 == 0)

    nc.compile()
    return nc


def _mblk():
    mb = np.zeros((8, 256), np.float32)
    for h in range(8):
        mb[h, h * 32 : (h + 1) * 32] = 1.0
    return mb.astype(ml_dtypes.bfloat16)


def _prep_inputs(feat0, zone_mask, w_q, w_k, w_v, conv1_w, bn1_g, bn1_b,
                 conv2_w, bn2_g, bn2_b, num_inside):
    B = feat0.shape[0]
    pos = np.asarray(zone_mask[:, :, 0])
    order = np.argsort(~pos, axis=1, kind="stable")
    assert np.array_equal(
        order[:, :num_inside],
        np.broadcast_to(np.arange(num_inside), (B, num_inside)),
    ), "kernel assumes inside positions are the first num_inside rows"
    assert num_inside == NI

    bf = ml_dtypes.bfloat16
    f32 = np.float32

    def wt(w):  # [dout, din] -> [128, 2, dout]: [p, ki, o] = w[o, ki*128+p]
        return np.ascontiguousarray(
            w.T.reshape(2, 128, D).transpose(1, 0, 2)
        ).astype(bf)

    def cw(w, nchunk):  # [O, I, 3, 3] -> [128, 9*nchunk, O]
        o_, i_, _, _ = w.shape
        r = w.transpose(2, 3, 1, 0).reshape(9, nchunk, 128, o_)
        return np.ascontiguousarray(
            r.transpose(2, 0, 1, 3).reshape(128, 9 * nchunk, o_)
        ).astype(bf)

    # conv1: f-channel taps (chunks 0,1) in rows 0..17, t-channel in 18..35
    c1 = np.asarray(conv1_w, f32)
    c1w_host = np.concatenate([cw(c1[:, :256], 2), cw(c1[:, 256:], 2)], axis=1)

    common = {
        "wqt": wt(np.asarray(w_q, f32)),
        "wkt": wt(np.asarray(w_k, f32)),
        "wvt": wt(np.asarray(w_v, f32)),
        "c1w": c1w_host,
        "c2w": cw(np.asarray(conv2_w, f32), 2),
        "bn1g": np.asarray(bn1_g, f32).reshape(D, 1),
        "bn1b": np.asarray(bn1_b, f32).reshape(D, 1),
        "bn2g": np.asarray(bn2_g, f32).reshape(D, 1),
        "bn2b": np.asarray(bn2_b, f32).reshape(D, 1),
        "mblk": _mblk(),
    }
    in_maps = []
    for b in range(NCORES):
        m = dict(common)
        m["ft"] = np.ascontiguousarray(np.asarray(feat0[b], f32).T).astype(bf)
        in_maps.append(m)
    return in_maps


def kernel(feat0, zone_mask, w_q, w_k, w_v, conv1_w, bn1_g, bn1_b,
           conv2_w, bn2_g, bn2_b, H=80, W=80, B=8, D=256, num_inside=3200,
           **_ignored):
    global LAST_EXEC_NS, LAST_MEAN_EXEC_NS
    if "nc" not in _cache:
        _cache["nc"] = build_nc()
    nc = _cache["nc"]

    in_maps = _prep_inputs(feat0, zone_mask, w_q, w_k, w_v, conv1_w, bn1_g,
                           bn1_b, conv2_w, bn2_g, bn2_b, int(num_inside))
    trace = os.environ.get("KERNEL_TRACE", "0") == "1"
    res = run_bass_kernel_spmd(nc, in_maps, list(range(NCORES)), trace=trace)
    LAST_EXEC_NS = res.exec_time_ns
    LAST_MEAN_EXEC_NS = res.mean_exec_time_ns
    out = np.empty((NCORES, HW, 256), np.float32)
    for b in range(NCORES):
        out[b] = res.results[b]["out_t"].T.astype(np.float32)
    return out
